# revision 36
# baseline (speedup 1.0000x reference)
"""Block-sparse attention TRN2 kernel (8 NeuronCores, SPMD over batch*heads).

Contract: kernel(**inputs) takes FULL unsharded inputs
  query/key/value: (2, 16, 2048, 128) f32, block_mask: (16, 16) bool,
  block_size: 128
and returns the FULL (2, 16, 2048, 128) f32 output.

Math per (b, h): for each 128x128 block pair (i, j) with block_mask[i, j]:
  A_ij = softmax(Q_i K_j^T / sqrt(128)) (softmax per block over k, no
  cross-block merge), O_i = sum_j A_ij V_j.

Device pipeline ([k, q] orientation; all matmul operands f16):
  Blocks are ordered by (row-group g = i//4, key block j, i) and packed
  8-per-chunk into [128, 1024] PSUM score tiles. The (g, j, i) order
  makes same-j blocks adjacent, so MM1 batches consecutive-i runs into
  single matmuls and pairs leftover singles via 3-level APs (fewer,
  longer PE instructions -> less per-matmul overhead, better p-state).
  Per chunk:
    MM1   S^T = KT_j^T @ QT_i per run (f16, PSUM f32)
    exp   one ACT op per chunk (PSUM f32 -> SBUF f16)
    d     ones[128,128] @ E -> denominators replicated across all 128
          partitions, written into the dead score tile's PSUM banks
    rmul  ONE fused custom-DVE op eh = E * approx_recip(d)
    MM2   O_i^T += V_j^T.T @ eh, batched over uniform-flag row runs,
          accumulated in a per-row-group [128, 512] PSUM tile (4 rows),
          drained to f16 via ACT when the group's last chunk completes.
  Emission is software-pipelined (MM1(c+2) / d(c+1)+rmul(c+1) / MM2(c))
  so no engine stream ever sits directly behind a cross-engine dep.
  Host does f16 packing and the final O^T -> O transpose.
"""

import math

import numpy as np

_RECIP_MUL = None


def _register_recip_mul():
    """Register a fused out = in0 * (1/in1) custom DVE op (one ~1 elem/cycle
    DVE pass; hardware has no tensor_tensor divide). Seed + one Newton pass,
    ~0.2% max rel err on the reciprocal."""
    global _RECIP_MUL
    if _RECIP_MUL is not None:
        return _RECIP_MUL
    import concourse.dve_ops as dve_ops

    NAME = "RECIP_MUL_ANT"
    for op in dve_ops.OPS:
        if op.name == NAME:
            _RECIP_MUL = op
            return op
    from concourse.dve_spec import AluOp, Bin, C0, C1, Spec, Src0, Src1, _has_src1, lower
    from concourse.dve_uop import DveOpSpec

    _not_x = Bin(AluOp.BITWISE_NOT, Src1, Src1)
    _y0 = _not_x * C0

    def _ref(in0, in1, c0, c1, c2):
        not_x = (~in1.astype(np.float32).view(np.int32)).view(np.float32)
        y0 = not_x * c0
        return (in0 * y0) * (c1 - in1 * y0)

    spec = Spec(body=(Src0 * _y0) * (C1 - Src1 * _y0), reference=_ref)
    row = dve_ops._CUSTOM_DVE_ROW_BASE + len(dve_ops.OPS)
    shas = {}
    for ver in ("v3", "v4"):
        s = DveOpSpec(
            name=NAME, opcode=row, uops=lower(spec, ver=ver), rd1_en=_has_src1(spec)
        )
        shas[ver] = s.sha(ver)
    op = dve_ops.DveOp(NAME, spec, subdim=False, uops_sha=shas)
    dve_ops.OPS.append(op)
    dve_ops.CUSTOM_DVE_SPECS[NAME] = spec
    dve_ops._SUB_OPCODE_FOR_NAME[NAME] = row
    _RECIP_MUL = op
    return op


B, H, S, D = 2, 16, 2048, 128
BS = 128
NB = S // BS
N_CORES = 8
N_HEADS = B * H
HPC = N_HEADS // N_CORES  # heads per core
CAP = 8  # blocks per chunk (8 * 128 = 1024 cols = 2 PSUM banks)
GR = 4  # rows per output group (4 * 128 f32 = one PSUM bank)
SCALE = 1.0 / math.sqrt(float(D))


def _plan(mask):
    """Group-major schedule.

    Returns a list of chunk dicts:
      g      row group (i // GR)
      used   columns used in the [128, CAP*BS] score tile
      mm1s   (off, [qoff] | [qoff1, qoff2], width, j)
      mm2s   (out_col, width, off, j, start, stop)
      open_group / close_group: bool (first / last chunk of the group)
      empty_rows: rows of g with no active blocks (only on open chunks)
    """
    mask = np.asarray(mask).astype(bool)
    assert mask.shape == (NB, NB)
    minj = {i: None for i in range(NB)}
    maxj = {i: None for i in range(NB)}
    for i in range(NB):
        js = np.flatnonzero(mask[i])
        if len(js):
            minj[i], maxj[i] = int(js[0]), int(js[-1])
    chunks = []
    for g in range(NB // GR):
        rows = range(g * GR, (g + 1) * GR)
        entries = []  # (j, i)
        for j in range(NB):
            for i in rows:
                if mask[i, j]:
                    entries.append((j, i))
        if not entries:
            continue
        # items per j: maximal consecutive-i runs
        items = []  # (j, i0, ln), chunk-orderable
        for j in range(NB):
            ii = [i for i in rows if mask[i, j]]
            k = 0
            while k < len(ii):
                ln = 1
                while k + ln < len(ii) and ii[k + ln] == ii[k] + ln:
                    ln += 1
                items.append((j, ii[k], ln))
                k += ln
        # chunk the blocks, keeping each j's items in one chunk so same-j
        # singles can pair into one 256-col matmul via a 3-level AP
        jgroups = {}
        for j, i0, ln in items:
            jgroups.setdefault(j, []).append((j, i0, ln))
        per_chunk = [[]]
        room = CAP
        for j in sorted(jgroups):
            jsz = sum(it[2] for it in jgroups[j])
            if jsz > room:
                per_chunk.append([])
                room = CAP
            per_chunk[-1].extend(jgroups[j])
            room -= jsz
        nch = len(per_chunk)
        for ci in range(nch):
            citems = per_chunk[ci]
            # order: per j, singles first (paired), then longer runs
            ordered = []
            for j in sorted({it[0] for it in citems}):
                sing = [it for it in citems if it[0] == j and it[2] == 1]
                long = [it for it in citems if it[0] == j and it[2] > 1]
                ordered.extend(sing)
                ordered.extend(long)
            mm1s = []
            mm2s = []
            off = 0
            a = 0
            while a < len(ordered):
                j, i0, ln = ordered[a]
                if (
                    ln == 1
                    and a + 1 < len(ordered)
                    and ordered[a + 1][0] == j
                    and ordered[a + 1][2] == 1
                    and off % (4 * BS) != 3 * BS
                ):
                    j2, i2, _ = ordered[a + 1]
                    mm1s.append((off, [i0 * BS, i2 * BS], 2 * BS, j))
                    for bi, ii_ in enumerate((i0, i2)):
                        mm2s.append(
                            ((ii_ - g * GR) * BS, BS, off + bi * BS, j, minj[ii_] == j)
                        )
                    off += 2 * BS
                    a += 2
                    continue
                # run (or lone single): split MM1 at 512-col bank boundaries
                ro, rq, rw = off, i0 * BS, ln * BS
                while rw > 0:
                    w = min(rw, 4 * BS - ro % (4 * BS))
                    mm1s.append((ro, [rq], w, j))
                    ro += w
                    rq += w
                    rw -= w
                # MM2: split run where first-touch status flips (a matmul's
                # bytes must be uniformly overwrite-pending or accumulate)
                s0 = 0
                while s0 < ln:
                    ft = minj[i0 + s0] == j
                    s1 = s0 + 1
                    while s1 < ln and (minj[i0 + s1] == j) == ft:
                        s1 += 1
                    mm2s.append(
                        ((i0 + s0 - g * GR) * BS, (s1 - s0) * BS, off + s0 * BS, j, ft)
                    )
                    s0 = s1
                off += ln * BS
                a += 1
            chunks.append(
                dict(
                    g=g,
                    used=off,
                    mm1s=mm1s,
                    mm2s=mm2s,
                    open_group=(ci == 0),
                    close_group=(ci == nch - 1),
                    empty_rows=[
                        i for i in rows if minj[i] is None
                    ]
                    if ci == 0
                    else [],
                )
            )
    return chunks


def _build(mask):
    import concourse.bacc as bacc
    import concourse.bass as bass
    import concourse.tile as tile
    from concourse import bass_isa, mybir
    from concourse.dve_ops import RECIP_APPROX_FAST_CONSTS as _RC

    f32 = mybir.dt.float32
    f16 = mybir.dt.float16
    AF = mybir.ActivationFunctionType

    recip_mul = _register_recip_mul()
    chunks = _plan(mask)
    n = len(chunks)

    nc = bacc.Bacc(
        "TRN2",
        target_bir_lowering=False,
        debug=False,
        enable_asserts=False,
        num_devices=N_CORES,
    )
    qt_d = nc.dram_tensor("qt", (HPC, D, S), f16, kind="ExternalInput").ap()
    kt_d = nc.dram_tensor("kt", (HPC, D, S), f16, kind="ExternalInput").ap()
    v_d = nc.dram_tensor("v", (HPC, BS, NB * BS), f16, kind="ExternalInput").ap()
    ot_d = nc.dram_tensor("ot", (HPC, D, S), f16, kind="ExternalOutput").ap()

    with tile.TileContext(nc) as tc:
        with (
            tc.tile_pool(name="heads", bufs=HPC) as heads,
            tc.tile_pool(name="const", bufs=1) as const,
            tc.tile_pool(name="e", bufs=4) as epool,
            tc.tile_pool(name="eh", bufs=4) as ehpool,
            tc.tile_pool(name="dsb", bufs=2) as dpool,
            tc.tile_pool(name="outp", bufs=4) as outpool,
            tc.tile_pool(name="ps_s", bufs=3, space="PSUM") as ps_s,
            tc.tile_pool(name="ps_o", bufs=2, space="PSUM") as ps_o,
        ):
            ones_t = const.tile([BS, BS], f16)
            nc.vector.memset(ones_t[:], 1.0)

            state = {}

            def emit_mm1(c):
                ch = chunks[c]
                s_ps = ps_s.tile([BS, CAP * BS], f32)
                for off, qoffs, w, j in ch["mm1s"]:
                    if len(qoffs) == 2:
                        base = state["qt"][:, qoffs[0] : qoffs[0] + BS]
                        rhs = bass.AP(
                            tensor=base.tensor,
                            offset=base.offset,
                            ap=[base.ap[0], [qoffs[1] - qoffs[0], 2], [1, BS]],
                        )
                    else:
                        rhs = state["qt"][:, qoffs[0] : qoffs[0] + w]
                    nc.tensor.matmul(
                        s_ps[:, off : off + w],
                        lhsT=state["kt"][:, j * BS : (j + 1) * BS],
                        rhs=rhs,
                        start=True,
                        stop=True,
                    )
                e_t = epool.tile([BS, CAP * BS], f16, tag="e")
                nc.scalar.activation(
                    e_t[:, : ch["used"]], s_ps[:, : ch["used"]], AF.Exp, scale=SCALE
                )
                state[("e", c)] = e_t
                state[("s", c)] = s_ps

            def emit_dmm(c, on_pool):
                ch = chunks[c]
                used = ch["used"]
                e_t = state[("e", c)]
                s_ps = state.pop(("s", c))
                if on_pool:
                    # denominators on the otherwise-idle GPSIMD: partition
                    # all-reduce over E (SBUF->SBUF), freeing the PE pass
                    d_in = dpool.tile([BS, CAP * BS], f32, tag="dsb", name="d_in")
                    nc.gpsimd.partition_all_reduce(
                        d_in[:, :used], e_t[:, :used], channels=BS,
                        reduce_op=bass_isa.ReduceOp.add,
                    )
                else:
                    # denominators via ones-matmul, replicated to all
                    # partitions, written into the (dead) score tile's PSUM
                    # banks: 512-col pieces (a PSUM accumulation group must
                    # stay within one bank)
                    d_in = s_ps
                    for half in range(0, used, 4 * BS):
                        hi = min(used, half + 4 * BS)
                        nc.tensor.matmul(
                            d_in[:, half:hi],
                            lhsT=ones_t[:],
                            rhs=e_t[:, half:hi],
                            start=True,
                            stop=True,
                        )
                eh_t = ehpool.tile([BS, CAP * BS], f16, tag="eh")
                nc.vector._custom_dve(
                    recip_mul,
                    out=eh_t[:, :used],
                    in0=e_t[:, :used],
                    in1=d_in[:, :used],
                    s0=_RC["s0"],
                    s1=_RC["s1"],
                    imm2=_RC["imm2"],
                )
                state[("eh", c)] = eh_t

            def emit_mm2(c):
                ch = chunks[c]
                eh_t = state.pop(("eh", c))
                state.pop(("e", c))
                if ch["open_group"]:
                    state["o_ps"] = ps_o.tile(
                        [D, GR * BS], f32, tag="o", name="o_ps"
                    )
                    state["o_started"] = False
                    for i in ch["empty_rows"]:
                        nc.vector.memset(
                            state["o_ps"][:, (i % GR) * BS : (i % GR + 1) * BS], 0.0
                        )
                o_ps = state["o_ps"]
                for out_col, w, off, j, _ft in ch["mm2s"]:
                    # One start=True per group tile (marks the whole bank
                    # pending-zero); later matmuls overwrite their first-touch
                    # bytes and accumulate elsewhere. skip_group_check
                    # silences the sim's one-open-group-per-region tracker.
                    nc.tensor.matmul(
                        o_ps[:, out_col : out_col + w],
                        lhsT=state["v"][:, j * BS : (j + 1) * BS],
                        rhs=eh_t[:, off : off + w],
                        start=not state["o_started"],
                        stop=ch["close_group"],
                        skip_group_check=True,
                    )
                    state["o_started"] = True
                if ch["close_group"]:
                    g = ch["g"]
                    o_sb = outpool.tile([D, GR * BS], f16, tag="osb")
                    # alternate drains between ACT and DVE
                    if (state["h"] * 4 + g) % 2 == 0:
                        nc.scalar.copy(o_sb[:], o_ps[:])
                    else:
                        nc.vector.tensor_scalar_mul(o_sb[:], o_ps[:], 1.0)
                    nc.sync.dma_start(
                        out=ot_d[state["h"], :, g * GR * BS : (g + 1) * GR * BS],
                        in_=o_sb[:],
                    )

            # prefetch every head's inputs up front (SP queue runs ahead)
            intiles = []
            for h in range(HPC):
                qt_t = heads.tile([D, S], f16, tag="qt")
                nc.sync.dma_start(out=qt_t[:], in_=qt_d[h])
                kt_t = heads.tile([D, S], f16, tag="kt")
                nc.sync.dma_start(out=kt_t[:], in_=kt_d[h])
                v_t = heads.tile([BS, NB * BS], f16, tag="v")
                nc.sync.dma_start(out=v_t[:], in_=v_d[h])
                intiles.append((qt_t, kt_t, v_t))

            cglob = 0
            for h in range(HPC):
                state["qt"], state["kt"], state["v"] = intiles[h]
                state["h"] = h

                emit_mm1(0)
                if n > 1:
                    emit_mm1(1)
                for c in range(n):
                    emit_dmm(c, on_pool=(cglob % 4 == 3))
                    cglob += 1
                    if c + 2 < n:
                        emit_mm1(c + 2)
                    if c >= 1:
                        emit_mm2(c - 1)
                emit_mm2(n - 1)

    nc.finalize()
    return nc


_CACHE = {}


def _get_program(mask):
    key = np.asarray(mask).astype(bool).tobytes()
    if key not in _CACHE:
        _CACHE[key] = _build(mask)
    return _CACHE[key]


def _shard_inputs(query, key, value):
    q = np.ascontiguousarray(query, dtype=np.float32).reshape(N_HEADS, S, D)
    k = np.ascontiguousarray(key, dtype=np.float32).reshape(N_HEADS, S, D)
    v = np.ascontiguousarray(value, dtype=np.float32).reshape(N_HEADS, S, D)
    qt = np.ascontiguousarray(q.transpose(0, 2, 1).astype(np.float16))  # (32, D, S)
    kt = np.ascontiguousarray(k.transpose(0, 2, 1).astype(np.float16))
    v16 = np.ascontiguousarray(
        v.reshape(N_HEADS, NB, BS, D).transpose(0, 2, 1, 3).astype(np.float16)
    ).reshape(N_HEADS, BS, NB * BS)
    in_maps = []
    for c in range(N_CORES):
        sl = slice(c * HPC, (c + 1) * HPC)
        in_maps.append(
            {
                "qt": np.ascontiguousarray(qt[sl]),
                "kt": np.ascontiguousarray(kt[sl]),
                "v": np.ascontiguousarray(v16[sl]),
            }
        )
    return in_maps


def _unshard_output(results):
    ot = np.concatenate([r["ot"] for r in results], axis=0)  # (32, D, S) f16
    out = ot.astype(np.float32).transpose(0, 2, 1).reshape(B, H, S, D)
    return np.ascontiguousarray(out)


def kernel(query, key, value, block_mask, block_size, _trace=False):
    from concourse.bass_utils import run_bass_kernel_spmd

    assert int(block_size) == BS
    nc = _get_program(block_mask)
    in_maps = _shard_inputs(query, key, value)
    res = run_bass_kernel_spmd(nc, in_maps, core_ids=list(range(N_CORES)), trace=_trace)
    out = _unshard_output(res.results)
    if _trace:
        return out, res
    return out


# revision 38
# speedup vs baseline: 1.8002x; 1.8002x over previous
"""Block-sparse attention TRN2 kernel (8 NeuronCores, SPMD over batch*heads).

Contract: kernel(**inputs) takes FULL unsharded inputs
  query/key/value: (2, 16, 2048, 128) f32, block_mask: (16, 16) bool,
  block_size: 128
and returns the FULL (2, 16, 2048, 128) f32 output.

Math per (b, h): for each 128x128 block pair (i, j) with block_mask[i, j]:
  A_ij = softmax(Q_i K_j^T / sqrt(128)) (softmax per block over k, no
  cross-block merge), O_i = sum_j A_ij V_j.

Device pipeline ([k, q] orientation; all matmul operands f16):
  Blocks are ordered by (row-group g = i//4, key block j, i) and packed
  8-per-chunk into [128, 1024] PSUM score tiles. The (g, j, i) order
  makes same-j blocks adjacent, so MM1 batches consecutive-i runs into
  single matmuls and pairs leftover singles via 3-level APs (fewer,
  longer PE instructions -> less per-matmul overhead, better p-state).
  Per chunk:
    MM1   S^T = KT_j^T @ QT_i per run (f16, PSUM f32)
    exp   one ACT op per chunk (PSUM f32 -> SBUF f16)
    d     ones[128,128] @ E -> denominators replicated across all 128
          partitions, written into the dead score tile's PSUM banks
    rmul  ONE fused custom-DVE op eh = E * approx_recip(d)
    MM2   O_i^T += V_j^T.T @ eh, batched over uniform-flag row runs,
          accumulated in a per-row-group [128, 512] PSUM tile (4 rows),
          drained to f16 via ACT when the group's last chunk completes.
  Emission is software-pipelined (MM1(c+2) / d(c+1)+rmul(c+1) / MM2(c))
  so no engine stream ever sits directly behind a cross-engine dep.
  Host does f16 packing and the final O^T -> O transpose.
"""

import math

import numpy as np

_RECIP_MUL = None


def _register_recip_mul():
    """Register a fused out = in0 * (1/in1) custom DVE op (one ~1 elem/cycle
    DVE pass; hardware has no tensor_tensor divide). Seed + one Newton pass,
    ~0.2% max rel err on the reciprocal."""
    global _RECIP_MUL
    if _RECIP_MUL is not None:
        return _RECIP_MUL
    import concourse.dve_ops as dve_ops

    NAME = "RECIP_MUL_ANT"
    for op in dve_ops.OPS:
        if op.name == NAME:
            _RECIP_MUL = op
            return op
    from concourse.dve_spec import AluOp, Bin, C0, C1, Spec, Src0, Src1, _has_src1, lower
    from concourse.dve_uop import DveOpSpec

    _not_x = Bin(AluOp.BITWISE_NOT, Src1, Src1)
    _y0 = _not_x * C0

    def _ref(in0, in1, c0, c1, c2):
        not_x = (~in1.astype(np.float32).view(np.int32)).view(np.float32)
        y0 = not_x * c0
        return (in0 * y0) * (c1 - in1 * y0)

    spec = Spec(body=(Src0 * _y0) * (C1 - Src1 * _y0), reference=_ref)
    row = dve_ops._CUSTOM_DVE_ROW_BASE + len(dve_ops.OPS)
    shas = {}
    for ver in ("v3", "v4"):
        s = DveOpSpec(
            name=NAME, opcode=row, uops=lower(spec, ver=ver), rd1_en=_has_src1(spec)
        )
        shas[ver] = s.sha(ver)
    op = dve_ops.DveOp(NAME, spec, subdim=False, uops_sha=shas)
    dve_ops.OPS.append(op)
    dve_ops.CUSTOM_DVE_SPECS[NAME] = spec
    dve_ops._SUB_OPCODE_FOR_NAME[NAME] = row
    _RECIP_MUL = op
    return op


B, H, S, D = 2, 16, 2048, 128
BS = 128
NB = S // BS
N_CORES = 8
N_HEADS = B * H
HPC = N_HEADS // N_CORES  # heads per core
CAP = 8  # blocks per chunk (8 * 128 = 1024 cols = 2 PSUM banks)
GR = 4  # rows per output group (4 * 128 f32 = one PSUM bank)
SCALE = 1.0 / math.sqrt(float(D))


def _plan(mask):
    """Group-major schedule.

    Returns a list of chunk dicts:
      g      row group (i // GR)
      used   columns used in the [128, CAP*BS] score tile
      mm1s   (off, [qoff] | [qoff1, qoff2], width, j)
      mm2s   (out_col, width, off, j, start, stop)
      open_group / close_group: bool (first / last chunk of the group)
      empty_rows: rows of g with no active blocks (only on open chunks)
    """
    mask = np.asarray(mask).astype(bool)
    assert mask.shape == (NB, NB)
    minj = {i: None for i in range(NB)}
    maxj = {i: None for i in range(NB)}
    for i in range(NB):
        js = np.flatnonzero(mask[i])
        if len(js):
            minj[i], maxj[i] = int(js[0]), int(js[-1])
    chunks = []
    for g in range(NB // GR):
        rows = range(g * GR, (g + 1) * GR)
        entries = []  # (j, i)
        for j in range(NB):
            for i in rows:
                if mask[i, j]:
                    entries.append((j, i))
        if not entries:
            continue
        # items per j: maximal consecutive-i runs
        items = []  # (j, i0, ln), chunk-orderable
        for j in range(NB):
            ii = [i for i in rows if mask[i, j]]
            k = 0
            while k < len(ii):
                ln = 1
                while k + ln < len(ii) and ii[k + ln] == ii[k] + ln:
                    ln += 1
                items.append((j, ii[k], ln))
                k += ln
        # chunk the blocks, keeping each j's items in one chunk so same-j
        # singles can pair into one 256-col matmul via a 3-level AP
        jgroups = {}
        for j, i0, ln in items:
            jgroups.setdefault(j, []).append((j, i0, ln))
        per_chunk = [[]]
        room = CAP
        for j in sorted(jgroups):
            jsz = sum(it[2] for it in jgroups[j])
            if jsz > room:
                per_chunk.append([])
                room = CAP
            per_chunk[-1].extend(jgroups[j])
            room -= jsz
        nch = len(per_chunk)
        for ci in range(nch):
            citems = per_chunk[ci]
            # order: per j, singles first (paired), then longer runs
            ordered = []
            for j in sorted({it[0] for it in citems}):
                sing = [it for it in citems if it[0] == j and it[2] == 1]
                long = [it for it in citems if it[0] == j and it[2] > 1]
                ordered.extend(sing)
                ordered.extend(long)
            mm1s = []
            mm2s = []
            off = 0
            a = 0
            while a < len(ordered):
                j, i0, ln = ordered[a]
                if (
                    ln == 1
                    and a + 1 < len(ordered)
                    and ordered[a + 1][0] == j
                    and ordered[a + 1][2] == 1
                    and off % (4 * BS) != 3 * BS
                ):
                    j2, i2, _ = ordered[a + 1]
                    mm1s.append((off, [i0 * BS, i2 * BS], 2 * BS, j))
                    for bi, ii_ in enumerate((i0, i2)):
                        mm2s.append(
                            ((ii_ - g * GR) * BS, BS, off + bi * BS, j, minj[ii_] == j)
                        )
                    off += 2 * BS
                    a += 2
                    continue
                # run (or lone single): split MM1 at 512-col bank boundaries
                ro, rq, rw = off, i0 * BS, ln * BS
                while rw > 0:
                    w = min(rw, 4 * BS - ro % (4 * BS))
                    mm1s.append((ro, [rq], w, j))
                    ro += w
                    rq += w
                    rw -= w
                # MM2: split run where first-touch status flips (a matmul's
                # bytes must be uniformly overwrite-pending or accumulate)
                s0 = 0
                while s0 < ln:
                    ft = minj[i0 + s0] == j
                    s1 = s0 + 1
                    while s1 < ln and (minj[i0 + s1] == j) == ft:
                        s1 += 1
                    mm2s.append(
                        ((i0 + s0 - g * GR) * BS, (s1 - s0) * BS, off + s0 * BS, j, ft)
                    )
                    s0 = s1
                off += ln * BS
                a += 1
            chunks.append(
                dict(
                    g=g,
                    used=off,
                    mm1s=mm1s,
                    mm2s=mm2s,
                    open_group=(ci == 0),
                    close_group=(ci == nch - 1),
                    empty_rows=[
                        i for i in rows if minj[i] is None
                    ]
                    if ci == 0
                    else [],
                )
            )
    return chunks


def _build(mask):
    import concourse.bacc as bacc
    import concourse.bass as bass
    import concourse.tile as tile
    from concourse import bass_isa, mybir
    from concourse.dve_ops import RECIP_APPROX_FAST_CONSTS as _RC

    f32 = mybir.dt.float32
    f16 = mybir.dt.float16
    AF = mybir.ActivationFunctionType

    recip_mul = _register_recip_mul()
    chunks = _plan(mask)
    n = len(chunks)

    nc = bacc.Bacc(
        "TRN2",
        target_bir_lowering=False,
        debug=False,
        enable_asserts=False,
        num_devices=N_CORES,
    )
    qt_d = nc.dram_tensor("qt", (HPC, D, S), f16, kind="ExternalInput").ap()
    kt_d = nc.dram_tensor("kt", (HPC, D, S), f16, kind="ExternalInput").ap()
    v_d = nc.dram_tensor("v", (HPC, BS, NB * BS), f16, kind="ExternalInput").ap()
    ot_d = nc.dram_tensor("ot", (HPC, D, S), f16, kind="ExternalOutput").ap()

    with tile.TileContext(nc) as tc:
        with (
            tc.tile_pool(name="heads", bufs=HPC) as heads,
            tc.tile_pool(name="const", bufs=1) as const,
            tc.tile_pool(name="e", bufs=4) as epool,
            tc.tile_pool(name="eh", bufs=4) as ehpool,
            tc.tile_pool(name="dsb", bufs=2) as dpool,
            tc.tile_pool(name="outp", bufs=4) as outpool,
            tc.tile_pool(name="ps_s", bufs=3, space="PSUM") as ps_s,
            tc.tile_pool(name="ps_o", bufs=2, space="PSUM") as ps_o,
        ):
            ones_t = const.tile([BS, BS], f16)
            nc.vector.memset(ones_t[:], 1.0)

            state = {}

            def emit_mm1(c):
                ch = chunks[c]
                s_ps = ps_s.tile([BS, CAP * BS], f32)
                for off, qoffs, w, j in ch["mm1s"]:
                    if len(qoffs) == 2:
                        base = state["qt"][:, qoffs[0] : qoffs[0] + BS]
                        rhs = bass.AP(
                            tensor=base.tensor,
                            offset=base.offset,
                            ap=[base.ap[0], [qoffs[1] - qoffs[0], 2], [1, BS]],
                        )
                    else:
                        rhs = state["qt"][:, qoffs[0] : qoffs[0] + w]
                    nc.tensor.matmul(
                        s_ps[:, off : off + w],
                        lhsT=state["kt"][:, j * BS : (j + 1) * BS],
                        rhs=rhs,
                        start=True,
                        stop=True,
                    )
                e_t = epool.tile([BS, CAP * BS], f16, tag="e")
                nc.scalar.activation(
                    e_t[:, : ch["used"]], s_ps[:, : ch["used"]], AF.Exp, scale=SCALE
                )
                state[("e", c)] = e_t
                state[("s", c)] = s_ps

            def emit_dmm(c, on_pool):
                ch = chunks[c]
                used = ch["used"]
                e_t = state[("e", c)]
                s_ps = state.pop(("s", c))
                # denominators via ones-matmul, replicated to all partitions,
                # written into the (dead) score tile's PSUM banks: 512-col
                # pieces (a PSUM accumulation group must stay within one bank)
                d_in = s_ps
                for half in range(0, used, 4 * BS):
                    hi = min(used, half + 4 * BS)
                    nc.tensor.matmul(
                        d_in[:, half:hi],
                        lhsT=ones_t[:],
                        rhs=e_t[:, half:hi],
                        start=True,
                        stop=True,
                    )
                eh_t = ehpool.tile([BS, CAP * BS], f16, tag="eh")
                nc.vector._custom_dve(
                    recip_mul,
                    out=eh_t[:, :used],
                    in0=e_t[:, :used],
                    in1=d_in[:, :used],
                    s0=_RC["s0"],
                    s1=_RC["s1"],
                    imm2=_RC["imm2"],
                )
                state[("eh", c)] = eh_t

            def emit_mm2(c):
                ch = chunks[c]
                eh_t = state.pop(("eh", c))
                state.pop(("e", c))
                if ch["open_group"]:
                    state["o_ps"] = ps_o.tile(
                        [D, GR * BS], f32, tag="o", name="o_ps"
                    )
                    state["o_started"] = False
                    for i in ch["empty_rows"]:
                        nc.vector.memset(
                            state["o_ps"][:, (i % GR) * BS : (i % GR + 1) * BS], 0.0
                        )
                o_ps = state["o_ps"]
                for out_col, w, off, j, _ft in ch["mm2s"]:
                    # One start=True per group tile (marks the whole bank
                    # pending-zero); later matmuls overwrite their first-touch
                    # bytes and accumulate elsewhere. skip_group_check
                    # silences the sim's one-open-group-per-region tracker.
                    nc.tensor.matmul(
                        o_ps[:, out_col : out_col + w],
                        lhsT=state["v"][:, j * BS : (j + 1) * BS],
                        rhs=eh_t[:, off : off + w],
                        start=not state["o_started"],
                        stop=ch["close_group"],
                        skip_group_check=True,
                    )
                    state["o_started"] = True
                if ch["close_group"]:
                    g = ch["g"]
                    o_sb = outpool.tile([D, GR * BS], f16, tag="osb")
                    # alternate drains between ACT and DVE
                    if (state["h"] * 4 + g) % 2 == 0:
                        nc.scalar.copy(o_sb[:], o_ps[:])
                    else:
                        nc.vector.tensor_scalar_mul(o_sb[:], o_ps[:], 1.0)
                    nc.sync.dma_start(
                        out=ot_d[state["h"], :, g * GR * BS : (g + 1) * GR * BS],
                        in_=o_sb[:],
                    )

            # prefetch every head's inputs up front (SP queue runs ahead)
            intiles = []
            for h in range(HPC):
                qt_t = heads.tile([D, S], f16, tag="qt")
                nc.sync.dma_start(out=qt_t[:], in_=qt_d[h])
                kt_t = heads.tile([D, S], f16, tag="kt")
                nc.sync.dma_start(out=kt_t[:], in_=kt_d[h])
                v_t = heads.tile([BS, NB * BS], f16, tag="v")
                nc.sync.dma_start(out=v_t[:], in_=v_d[h])
                intiles.append((qt_t, kt_t, v_t))

            cglob = 0
            for h in range(HPC):
                state["qt"], state["kt"], state["v"] = intiles[h]
                state["h"] = h

                emit_mm1(0)
                if n > 1:
                    emit_mm1(1)
                for c in range(n):
                    emit_dmm(c, on_pool=False)
                    cglob += 1
                    if c + 2 < n:
                        emit_mm1(c + 2)
                    if c >= 1:
                        emit_mm2(c - 1)
                emit_mm2(n - 1)

    nc.finalize()
    return nc


_CACHE = {}


def _get_program(mask):
    key = np.asarray(mask).astype(bool).tobytes()
    if key not in _CACHE:
        _CACHE[key] = _build(mask)
    return _CACHE[key]


def _shard_inputs(query, key, value):
    q = np.ascontiguousarray(query, dtype=np.float32).reshape(N_HEADS, S, D)
    k = np.ascontiguousarray(key, dtype=np.float32).reshape(N_HEADS, S, D)
    v = np.ascontiguousarray(value, dtype=np.float32).reshape(N_HEADS, S, D)
    qt = np.ascontiguousarray(q.transpose(0, 2, 1).astype(np.float16))  # (32, D, S)
    kt = np.ascontiguousarray(k.transpose(0, 2, 1).astype(np.float16))
    v16 = np.ascontiguousarray(
        v.reshape(N_HEADS, NB, BS, D).transpose(0, 2, 1, 3).astype(np.float16)
    ).reshape(N_HEADS, BS, NB * BS)
    in_maps = []
    for c in range(N_CORES):
        sl = slice(c * HPC, (c + 1) * HPC)
        in_maps.append(
            {
                "qt": np.ascontiguousarray(qt[sl]),
                "kt": np.ascontiguousarray(kt[sl]),
                "v": np.ascontiguousarray(v16[sl]),
            }
        )
    return in_maps


def _unshard_output(results):
    ot = np.concatenate([r["ot"] for r in results], axis=0)  # (32, D, S) f16
    out = ot.astype(np.float32).transpose(0, 2, 1).reshape(B, H, S, D)
    return np.ascontiguousarray(out)


def kernel(query, key, value, block_mask, block_size, _trace=False):
    from concourse.bass_utils import run_bass_kernel_spmd

    assert int(block_size) == BS
    nc = _get_program(block_mask)
    in_maps = _shard_inputs(query, key, value)
    res = run_bass_kernel_spmd(nc, in_maps, core_ids=list(range(N_CORES)), trace=_trace)
    out = _unshard_output(res.results)
    if _trace:
        return out, res
    return out


# revision 43
# speedup vs baseline: 1.8647x; 1.0358x over previous
"""Block-sparse attention TRN2 kernel (8 NeuronCores, SPMD over batch*heads).

Contract: kernel(**inputs) takes FULL unsharded inputs
  query/key/value: (2, 16, 2048, 128) f32, block_mask: (16, 16) bool,
  block_size: 128
and returns the FULL (2, 16, 2048, 128) f32 output.

Math per (b, h): for each 128x128 block pair (i, j) with block_mask[i, j]:
  A_ij = softmax(Q_i K_j^T / sqrt(128)) (softmax per block over k, no
  cross-block merge), O_i = sum_j A_ij V_j.

Device pipeline ([k, q] orientation; all matmul operands f16):
  Blocks are ordered by (row-group g = i//4, key block j, i) and packed
  8-per-chunk into [128, 1024] PSUM score tiles. The (g, j, i) order
  makes same-j blocks adjacent, so MM1 batches consecutive-i runs into
  single matmuls and pairs leftover singles via 3-level APs (fewer,
  longer PE instructions -> less per-matmul overhead, better p-state).
  Per chunk:
    MM1   S^T = KT_j^T @ QT_i per run (f16, PSUM f32)
    exp   one ACT op per chunk (PSUM f32 -> SBUF f16)
    d     ones[128,128] @ E -> denominators replicated across all 128
          partitions, written into the dead score tile's PSUM banks
    rmul  ONE fused custom-DVE op eh = E * approx_recip(d)
    MM2   O_i^T += V_j^T.T @ eh, batched over uniform-flag row runs,
          accumulated in a per-row-group [128, 512] PSUM tile (4 rows),
          drained to f16 via ACT when the group's last chunk completes.
  Emission is software-pipelined (MM1(c+2) / d(c+1)+rmul(c+1) / MM2(c))
  so no engine stream ever sits directly behind a cross-engine dep.
  Host does f16 packing and the final O^T -> O transpose.
"""

import math

import numpy as np

_RECIP_MUL = None


def _register_recip_mul():
    """Register a fused out = in0 * (1/in1) custom DVE op (one ~1 elem/cycle
    DVE pass; hardware has no tensor_tensor divide). Seed + one Newton pass,
    ~0.2% max rel err on the reciprocal."""
    global _RECIP_MUL
    if _RECIP_MUL is not None:
        return _RECIP_MUL
    import concourse.dve_ops as dve_ops

    NAME = "RECIP_MUL_ANT"
    for op in dve_ops.OPS:
        if op.name == NAME:
            _RECIP_MUL = op
            return op
    from concourse.dve_spec import AluOp, Bin, C0, C1, Spec, Src0, Src1, _has_src1, lower
    from concourse.dve_uop import DveOpSpec

    _not_x = Bin(AluOp.BITWISE_NOT, Src1, Src1)
    _y0 = _not_x * C0

    def _ref(in0, in1, c0, c1, c2):
        not_x = (~in1.astype(np.float32).view(np.int32)).view(np.float32)
        y0 = not_x * c0
        return (in0 * y0) * (c1 - in1 * y0)

    spec = Spec(body=(Src0 * _y0) * (C1 - Src1 * _y0), reference=_ref)
    row = dve_ops._CUSTOM_DVE_ROW_BASE + len(dve_ops.OPS)
    shas = {}
    for ver in ("v3", "v4"):
        s = DveOpSpec(
            name=NAME, opcode=row, uops=lower(spec, ver=ver), rd1_en=_has_src1(spec)
        )
        shas[ver] = s.sha(ver)
    op = dve_ops.DveOp(NAME, spec, subdim=False, uops_sha=shas)
    dve_ops.OPS.append(op)
    dve_ops.CUSTOM_DVE_SPECS[NAME] = spec
    dve_ops._SUB_OPCODE_FOR_NAME[NAME] = row
    _RECIP_MUL = op
    return op


B, H, S, D = 2, 16, 2048, 128
BS = 128
NB = S // BS
N_CORES = 8
N_HEADS = B * H
HPC = N_HEADS // N_CORES  # heads per core
CAP = 8  # blocks per chunk (8 * 128 = 1024 cols = 2 PSUM banks)
GR = 4  # rows per output group (4 * 128 f32 = one PSUM bank)
SCALE = 1.0 / math.sqrt(float(D))


def _plan(mask):
    """Group-major schedule.

    Returns a list of chunk dicts:
      g      row group (i // GR)
      used   columns used in the [128, CAP*BS] score tile
      mm1s   (off, [qoff] | [qoff1, qoff2], width, j)
      mm2s   (out_col, width, off, j, start, stop)
      open_group / close_group: bool (first / last chunk of the group)
      empty_rows: rows of g with no active blocks (only on open chunks)
    """
    mask = np.asarray(mask).astype(bool)
    assert mask.shape == (NB, NB)
    minj = {i: None for i in range(NB)}
    maxj = {i: None for i in range(NB)}
    for i in range(NB):
        js = np.flatnonzero(mask[i])
        if len(js):
            minj[i], maxj[i] = int(js[0]), int(js[-1])
    chunks = []
    for g in range(NB // GR):
        rows = range(g * GR, (g + 1) * GR)
        entries = []  # (j, i)
        for j in range(NB):
            for i in rows:
                if mask[i, j]:
                    entries.append((j, i))
        if not entries:
            continue
        # items per j: maximal consecutive-i runs
        items = []  # (j, i0, ln), chunk-orderable
        for j in range(NB):
            ii = [i for i in rows if mask[i, j]]
            k = 0
            while k < len(ii):
                ln = 1
                while k + ln < len(ii) and ii[k + ln] == ii[k] + ln:
                    ln += 1
                items.append((j, ii[k], ln))
                k += ln
        # chunk the blocks, keeping each j's items in one chunk so same-j
        # singles can pair into one 256-col matmul via a 3-level AP
        jgroups = {}
        for j, i0, ln in items:
            jgroups.setdefault(j, []).append((j, i0, ln))
        per_chunk = [[]]
        room = CAP
        for j in sorted(jgroups):
            jsz = sum(it[2] for it in jgroups[j])
            if jsz > room:
                per_chunk.append([])
                room = CAP
            per_chunk[-1].extend(jgroups[j])
            room -= jsz
        nch = len(per_chunk)
        for ci in range(nch):
            citems = per_chunk[ci]
            # order: per j, singles first (paired), then longer runs
            ordered = []
            for j in sorted({it[0] for it in citems}):
                sing = [it for it in citems if it[0] == j and it[2] == 1]
                long = [it for it in citems if it[0] == j and it[2] > 1]
                ordered.extend(sing)
                ordered.extend(long)
            mm1s = []
            mm2s = []
            off = 0
            a = 0
            while a < len(ordered):
                j, i0, ln = ordered[a]
                if (
                    ln == 1
                    and a + 1 < len(ordered)
                    and ordered[a + 1][0] == j
                    and ordered[a + 1][2] == 1
                    and off % (4 * BS) != 3 * BS
                ):
                    j2, i2, _ = ordered[a + 1]
                    mm1s.append((off, [i0 * BS, i2 * BS], 2 * BS, j))
                    for bi, ii_ in enumerate((i0, i2)):
                        mm2s.append(
                            ((ii_ - g * GR) * BS, BS, off + bi * BS, j, minj[ii_] == j)
                        )
                    off += 2 * BS
                    a += 2
                    continue
                # run (or lone single): split MM1 at 512-col bank boundaries
                ro, rq, rw = off, i0 * BS, ln * BS
                while rw > 0:
                    w = min(rw, 4 * BS - ro % (4 * BS))
                    mm1s.append((ro, [rq], w, j))
                    ro += w
                    rq += w
                    rw -= w
                # MM2: split run where first-touch status flips (a matmul's
                # bytes must be uniformly overwrite-pending or accumulate)
                s0 = 0
                while s0 < ln:
                    ft = minj[i0 + s0] == j
                    s1 = s0 + 1
                    while s1 < ln and (minj[i0 + s1] == j) == ft:
                        s1 += 1
                    mm2s.append(
                        ((i0 + s0 - g * GR) * BS, (s1 - s0) * BS, off + s0 * BS, j, ft)
                    )
                    s0 = s1
                off += ln * BS
                a += 1
            chunks.append(
                dict(
                    g=g,
                    used=off,
                    mm1s=mm1s,
                    mm2s=mm2s,
                    open_group=(ci == 0),
                    close_group=(ci == nch - 1),
                    empty_rows=[
                        i for i in rows if minj[i] is None
                    ]
                    if ci == 0
                    else [],
                )
            )
    return chunks


def _build(mask):
    import concourse.bacc as bacc
    import concourse.bass as bass
    import concourse.tile as tile
    from concourse import bass_isa, mybir
    from concourse.dve_ops import RECIP_APPROX_FAST_CONSTS as _RC

    f32 = mybir.dt.float32
    f16 = mybir.dt.float16
    AF = mybir.ActivationFunctionType

    recip_mul = _register_recip_mul()
    chunks = _plan(mask)
    n = len(chunks)

    nc = bacc.Bacc(
        "TRN2",
        target_bir_lowering=False,
        debug=False,
        enable_asserts=False,
        num_devices=N_CORES,
    )
    qt_d = nc.dram_tensor("qt", (HPC, D, S), f16, kind="ExternalInput").ap()
    kt_d = nc.dram_tensor("kt", (HPC, D, S), f16, kind="ExternalInput").ap()
    v_d = nc.dram_tensor("v", (HPC, BS, NB * BS), f16, kind="ExternalInput").ap()
    ot_d = nc.dram_tensor("ot", (HPC, D, S), f16, kind="ExternalOutput").ap()

    with tile.TileContext(nc) as tc:
        with (
            tc.tile_pool(name="heads", bufs=HPC) as heads,
            tc.tile_pool(name="const", bufs=1) as const,
            tc.tile_pool(name="e", bufs=4) as epool,
            tc.tile_pool(name="eh", bufs=4) as ehpool,
            tc.tile_pool(name="dsb", bufs=2) as dpool,
            tc.tile_pool(name="outp", bufs=4) as outpool,
            tc.tile_pool(name="ps_s", bufs=3, space="PSUM") as ps_s,
            tc.tile_pool(name="ps_o", bufs=2, space="PSUM") as ps_o,
        ):
            ones_t = const.tile([BS, BS], f16)
            nc.vector.memset(ones_t[:], 1.0)

            # PE warmup while the first input DMAs land: keeps the tensor
            # engine busy from t=0 so it ramps to full p-state
            warm_ps = ps_s.tile([BS, CAP * BS], f32, tag="s", name="warm_ps")
            for wi in range(40):
                nc.tensor.matmul(
                    warm_ps[:, (wi % 8) * BS : (wi % 8) * BS + BS],
                    lhsT=ones_t[:],
                    rhs=ones_t[:],
                    start=True,
                    stop=True,
                )

            state = {}

            def emit_mm1(c):
                ch = chunks[c]
                s_ps = ps_s.tile([BS, CAP * BS], f32, tag="s", name="s_ps")
                for off, qoffs, w, j in ch["mm1s"]:
                    if len(qoffs) == 2:
                        base = state["qt"][:, qoffs[0] : qoffs[0] + BS]
                        rhs = bass.AP(
                            tensor=base.tensor,
                            offset=base.offset,
                            ap=[base.ap[0], [qoffs[1] - qoffs[0], 2], [1, BS]],
                        )
                    else:
                        rhs = state["qt"][:, qoffs[0] : qoffs[0] + w]
                    nc.tensor.matmul(
                        s_ps[:, off : off + w],
                        lhsT=state["kt"][:, j * BS : (j + 1) * BS],
                        rhs=rhs,
                        start=True,
                        stop=True,
                    )
                e_t = epool.tile([BS, CAP * BS], f16, tag="e")
                nc.scalar.activation(
                    e_t[:, : ch["used"]], s_ps[:, : ch["used"]], AF.Exp, scale=SCALE
                )
                state[("e", c)] = e_t
                state[("s", c)] = s_ps

            def emit_dmm(c, on_pool):
                ch = chunks[c]
                used = ch["used"]
                e_t = state[("e", c)]
                s_ps = state.pop(("s", c))
                # denominators via ones-matmul, replicated to all partitions,
                # written into the (dead) score tile's PSUM banks: 512-col
                # pieces (a PSUM accumulation group must stay within one bank)
                d_in = s_ps
                for half in range(0, used, 4 * BS):
                    hi = min(used, half + 4 * BS)
                    nc.tensor.matmul(
                        d_in[:, half:hi],
                        lhsT=ones_t[:],
                        rhs=e_t[:, half:hi],
                        start=True,
                        stop=True,
                    )
                eh_t = ehpool.tile([BS, CAP * BS], f16, tag="eh")
                nc.vector._custom_dve(
                    recip_mul,
                    out=eh_t[:, :used],
                    in0=e_t[:, :used],
                    in1=d_in[:, :used],
                    s0=_RC["s0"],
                    s1=_RC["s1"],
                    imm2=_RC["imm2"],
                )
                state[("eh", c)] = eh_t

            def emit_mm2(c):
                ch = chunks[c]
                eh_t = state.pop(("eh", c))
                state.pop(("e", c))
                if ch["open_group"]:
                    state["o_ps"] = ps_o.tile(
                        [D, GR * BS], f32, tag="o", name="o_ps"
                    )
                    state["o_started"] = False
                    for i in ch["empty_rows"]:
                        nc.vector.memset(
                            state["o_ps"][:, (i % GR) * BS : (i % GR + 1) * BS], 0.0
                        )
                o_ps = state["o_ps"]
                for out_col, w, off, j, _ft in ch["mm2s"]:
                    # One start=True per group tile (marks the whole bank
                    # pending-zero); later matmuls overwrite their first-touch
                    # bytes and accumulate elsewhere. skip_group_check
                    # silences the sim's one-open-group-per-region tracker.
                    nc.tensor.matmul(
                        o_ps[:, out_col : out_col + w],
                        lhsT=state["v"][:, j * BS : (j + 1) * BS],
                        rhs=eh_t[:, off : off + w],
                        start=not state["o_started"],
                        stop=ch["close_group"],
                        skip_group_check=True,
                    )
                    state["o_started"] = True
                if ch["close_group"]:
                    g = ch["g"]
                    o_sb = outpool.tile([D, GR * BS], f16, tag="osb")
                    # alternate drains between ACT and DVE
                    if (state["h"] * 4 + g) % 2 == 0:
                        nc.scalar.copy(o_sb[:], o_ps[:])
                    else:
                        nc.vector.tensor_scalar_mul(o_sb[:], o_ps[:], 1.0)
                    nc.sync.dma_start(
                        out=ot_d[state["h"], :, g * GR * BS : (g + 1) * GR * BS],
                        in_=o_sb[:],
                    )

            # prefetch every head's inputs up front (SP queue runs ahead)
            intiles = []
            for h in range(HPC):
                qt_t = heads.tile([D, S], f16, tag="qt")
                nc.sync.dma_start(out=qt_t[:], in_=qt_d[h])
                kt_t = heads.tile([D, S], f16, tag="kt")
                nc.sync.dma_start(out=kt_t[:], in_=kt_d[h])
                v_t = heads.tile([BS, NB * BS], f16, tag="v")
                nc.sync.dma_start(out=v_t[:], in_=v_d[h])
                intiles.append((qt_t, kt_t, v_t))

            cglob = 0
            for h in range(HPC):
                state["qt"], state["kt"], state["v"] = intiles[h]
                state["h"] = h

                emit_mm1(0)
                if n > 1:
                    emit_mm1(1)
                for c in range(n):
                    emit_dmm(c, on_pool=False)
                    cglob += 1
                    if c + 2 < n:
                        emit_mm1(c + 2)
                    if c >= 2:
                        emit_mm2(c - 2)
                for c in (n - 2, n - 1):
                    if c >= 0:
                        emit_mm2(c)

    nc.finalize()
    return nc


_CACHE = {}


def _get_program(mask):
    key = np.asarray(mask).astype(bool).tobytes()
    if key not in _CACHE:
        _CACHE[key] = _build(mask)
    return _CACHE[key]


def _shard_inputs(query, key, value):
    q = np.ascontiguousarray(query, dtype=np.float32).reshape(N_HEADS, S, D)
    k = np.ascontiguousarray(key, dtype=np.float32).reshape(N_HEADS, S, D)
    v = np.ascontiguousarray(value, dtype=np.float32).reshape(N_HEADS, S, D)
    qt = np.ascontiguousarray(q.transpose(0, 2, 1).astype(np.float16))  # (32, D, S)
    kt = np.ascontiguousarray(k.transpose(0, 2, 1).astype(np.float16))
    v16 = np.ascontiguousarray(
        v.reshape(N_HEADS, NB, BS, D).transpose(0, 2, 1, 3).astype(np.float16)
    ).reshape(N_HEADS, BS, NB * BS)
    in_maps = []
    for c in range(N_CORES):
        sl = slice(c * HPC, (c + 1) * HPC)
        in_maps.append(
            {
                "qt": np.ascontiguousarray(qt[sl]),
                "kt": np.ascontiguousarray(kt[sl]),
                "v": np.ascontiguousarray(v16[sl]),
            }
        )
    return in_maps


def _unshard_output(results):
    ot = np.concatenate([r["ot"] for r in results], axis=0)  # (32, D, S) f16
    out = ot.astype(np.float32).transpose(0, 2, 1).reshape(B, H, S, D)
    return np.ascontiguousarray(out)


def kernel(query, key, value, block_mask, block_size, _trace=False):
    from concourse.bass_utils import run_bass_kernel_spmd

    assert int(block_size) == BS
    nc = _get_program(block_mask)
    in_maps = _shard_inputs(query, key, value)
    res = run_bass_kernel_spmd(nc, in_maps, core_ids=list(range(N_CORES)), trace=_trace)
    out = _unshard_output(res.results)
    if _trace:
        return out, res
    return out


# revision 44
# speedup vs baseline: 1.8669x; 1.0012x over previous
"""Block-sparse attention TRN2 kernel (8 NeuronCores, SPMD over batch*heads).

Contract: kernel(**inputs) takes FULL unsharded inputs
  query/key/value: (2, 16, 2048, 128) f32, block_mask: (16, 16) bool,
  block_size: 128
and returns the FULL (2, 16, 2048, 128) f32 output.

Math per (b, h): for each 128x128 block pair (i, j) with block_mask[i, j]:
  A_ij = softmax(Q_i K_j^T / sqrt(128)) (softmax per block over k, no
  cross-block merge), O_i = sum_j A_ij V_j.

Device pipeline ([k, q] orientation; all matmul operands f16):
  Blocks are ordered by (row-group g = i//4, key block j, i) and packed
  8-per-chunk into [128, 1024] PSUM score tiles. The (g, j, i) order
  makes same-j blocks adjacent, so MM1 batches consecutive-i runs into
  single matmuls and pairs leftover singles via 3-level APs (fewer,
  longer PE instructions -> less per-matmul overhead, better p-state).
  Per chunk:
    MM1   S^T = KT_j^T @ QT_i per run (f16, PSUM f32)
    exp   one ACT op per chunk (PSUM f32 -> SBUF f16)
    d     ones[128,128] @ E -> denominators replicated across all 128
          partitions, written into the dead score tile's PSUM banks
    rmul  ONE fused custom-DVE op eh = E * approx_recip(d)
    MM2   O_i^T += V_j^T.T @ eh, batched over uniform-flag row runs,
          accumulated in a per-row-group [128, 512] PSUM tile (4 rows),
          drained to f16 via ACT when the group's last chunk completes.
  Emission is software-pipelined (MM1(c+2) / d(c+1)+rmul(c+1) / MM2(c))
  so no engine stream ever sits directly behind a cross-engine dep.
  Host does f16 packing and the final O^T -> O transpose.
"""

import math

import numpy as np

_RECIP_MUL = None


def _register_recip_mul():
    """Register a fused out = in0 * (1/in1) custom DVE op (one ~1 elem/cycle
    DVE pass; hardware has no tensor_tensor divide). Seed + one Newton pass,
    ~0.2% max rel err on the reciprocal."""
    global _RECIP_MUL
    if _RECIP_MUL is not None:
        return _RECIP_MUL
    import concourse.dve_ops as dve_ops

    NAME = "RECIP_MUL_ANT"
    for op in dve_ops.OPS:
        if op.name == NAME:
            _RECIP_MUL = op
            return op
    from concourse.dve_spec import AluOp, Bin, C0, C1, Spec, Src0, Src1, _has_src1, lower
    from concourse.dve_uop import DveOpSpec

    _not_x = Bin(AluOp.BITWISE_NOT, Src1, Src1)
    _y0 = _not_x * C0

    def _ref(in0, in1, c0, c1, c2):
        not_x = (~in1.astype(np.float32).view(np.int32)).view(np.float32)
        y0 = not_x * c0
        return (in0 * y0) * (c1 - in1 * y0)

    spec = Spec(body=(Src0 * _y0) * (C1 - Src1 * _y0), reference=_ref)
    row = dve_ops._CUSTOM_DVE_ROW_BASE + len(dve_ops.OPS)
    shas = {}
    for ver in ("v3", "v4"):
        s = DveOpSpec(
            name=NAME, opcode=row, uops=lower(spec, ver=ver), rd1_en=_has_src1(spec)
        )
        shas[ver] = s.sha(ver)
    op = dve_ops.DveOp(NAME, spec, subdim=False, uops_sha=shas)
    dve_ops.OPS.append(op)
    dve_ops.CUSTOM_DVE_SPECS[NAME] = spec
    dve_ops._SUB_OPCODE_FOR_NAME[NAME] = row
    _RECIP_MUL = op
    return op


B, H, S, D = 2, 16, 2048, 128
BS = 128
NB = S // BS
N_CORES = 8
N_HEADS = B * H
HPC = N_HEADS // N_CORES  # heads per core
CAP = 8  # blocks per chunk (8 * 128 = 1024 cols = 2 PSUM banks)
GR = 4  # rows per output group (4 * 128 f32 = one PSUM bank)
SCALE = 1.0 / math.sqrt(float(D))


def _plan(mask):
    """Group-major schedule.

    Returns a list of chunk dicts:
      g      row group (i // GR)
      used   columns used in the [128, CAP*BS] score tile
      mm1s   (off, [qoff] | [qoff1, qoff2], width, j)
      mm2s   (out_col, width, off, j, start, stop)
      open_group / close_group: bool (first / last chunk of the group)
      empty_rows: rows of g with no active blocks (only on open chunks)
    """
    mask = np.asarray(mask).astype(bool)
    assert mask.shape == (NB, NB)
    minj = {i: None for i in range(NB)}
    maxj = {i: None for i in range(NB)}
    for i in range(NB):
        js = np.flatnonzero(mask[i])
        if len(js):
            minj[i], maxj[i] = int(js[0]), int(js[-1])
    chunks = []
    for g in range(NB // GR):
        rows = range(g * GR, (g + 1) * GR)
        entries = []  # (j, i)
        for j in range(NB):
            for i in rows:
                if mask[i, j]:
                    entries.append((j, i))
        if not entries:
            continue
        # items per j: maximal consecutive-i runs
        items = []  # (j, i0, ln), chunk-orderable
        for j in range(NB):
            ii = [i for i in rows if mask[i, j]]
            k = 0
            while k < len(ii):
                ln = 1
                while k + ln < len(ii) and ii[k + ln] == ii[k] + ln:
                    ln += 1
                items.append((j, ii[k], ln))
                k += ln
        # chunk the blocks, keeping each j's items in one chunk so same-j
        # singles can pair into one 256-col matmul via a 3-level AP
        jgroups = {}
        for j, i0, ln in items:
            jgroups.setdefault(j, []).append((j, i0, ln))
        per_chunk = [[]]
        room = CAP
        for j in sorted(jgroups):
            jsz = sum(it[2] for it in jgroups[j])
            if jsz > room:
                per_chunk.append([])
                room = CAP
            per_chunk[-1].extend(jgroups[j])
            room -= jsz
        nch = len(per_chunk)
        for ci in range(nch):
            citems = per_chunk[ci]
            # order: per j, singles first (paired), then longer runs
            ordered = []
            for j in sorted({it[0] for it in citems}):
                sing = [it for it in citems if it[0] == j and it[2] == 1]
                long = [it for it in citems if it[0] == j and it[2] > 1]
                ordered.extend(sing)
                ordered.extend(long)
            mm1s = []
            mm2s = []
            off = 0
            a = 0
            while a < len(ordered):
                j, i0, ln = ordered[a]
                if (
                    ln == 1
                    and a + 1 < len(ordered)
                    and ordered[a + 1][0] == j
                    and ordered[a + 1][2] == 1
                    and off % (4 * BS) != 3 * BS
                ):
                    j2, i2, _ = ordered[a + 1]
                    mm1s.append((off, [i0 * BS, i2 * BS], 2 * BS, j))
                    for bi, ii_ in enumerate((i0, i2)):
                        mm2s.append(
                            ((ii_ - g * GR) * BS, BS, off + bi * BS, j, minj[ii_] == j)
                        )
                    off += 2 * BS
                    a += 2
                    continue
                # run (or lone single): split MM1 at 512-col bank boundaries
                ro, rq, rw = off, i0 * BS, ln * BS
                while rw > 0:
                    w = min(rw, 4 * BS - ro % (4 * BS))
                    mm1s.append((ro, [rq], w, j))
                    ro += w
                    rq += w
                    rw -= w
                # MM2: split run where first-touch status flips (a matmul's
                # bytes must be uniformly overwrite-pending or accumulate)
                s0 = 0
                while s0 < ln:
                    ft = minj[i0 + s0] == j
                    s1 = s0 + 1
                    while s1 < ln and (minj[i0 + s1] == j) == ft:
                        s1 += 1
                    mm2s.append(
                        ((i0 + s0 - g * GR) * BS, (s1 - s0) * BS, off + s0 * BS, j, ft)
                    )
                    s0 = s1
                off += ln * BS
                a += 1
            chunks.append(
                dict(
                    g=g,
                    used=off,
                    mm1s=mm1s,
                    mm2s=mm2s,
                    open_group=(ci == 0),
                    close_group=(ci == nch - 1),
                    empty_rows=[
                        i for i in rows if minj[i] is None
                    ]
                    if ci == 0
                    else [],
                )
            )
    return chunks


def _build(mask):
    import concourse.bacc as bacc
    import concourse.bass as bass
    import concourse.tile as tile
    from concourse import bass_isa, mybir
    from concourse.dve_ops import RECIP_APPROX_FAST_CONSTS as _RC

    f32 = mybir.dt.float32
    f16 = mybir.dt.float16
    AF = mybir.ActivationFunctionType

    recip_mul = _register_recip_mul()
    chunks = _plan(mask)
    n = len(chunks)

    nc = bacc.Bacc(
        "TRN2",
        target_bir_lowering=False,
        debug=False,
        enable_asserts=False,
        num_devices=N_CORES,
    )
    qt_d = nc.dram_tensor("qt", (HPC, D, S), f16, kind="ExternalInput").ap()
    kt_d = nc.dram_tensor("kt", (HPC, D, S), f16, kind="ExternalInput").ap()
    v_d = nc.dram_tensor("v", (HPC, BS, NB * BS), f16, kind="ExternalInput").ap()
    ot_d = nc.dram_tensor("ot", (HPC, D, S), f16, kind="ExternalOutput").ap()

    with tile.TileContext(nc) as tc:
        with (
            tc.tile_pool(name="heads", bufs=HPC) as heads,
            tc.tile_pool(name="const", bufs=1) as const,
            tc.tile_pool(name="e", bufs=6) as epool,
            tc.tile_pool(name="eh", bufs=6) as ehpool,
            tc.tile_pool(name="dsb", bufs=2) as dpool,
            tc.tile_pool(name="outp", bufs=4) as outpool,
            tc.tile_pool(name="ps_s", bufs=3, space="PSUM") as ps_s,
            tc.tile_pool(name="ps_o", bufs=2, space="PSUM") as ps_o,
        ):
            ones_t = const.tile([BS, BS], f16)
            nc.vector.memset(ones_t[:], 1.0)

            # PE warmup while the first input DMAs land: keeps the tensor
            # engine busy from t=0 so it ramps to full p-state
            warm_ps = ps_s.tile([BS, CAP * BS], f32, tag="s", name="warm_ps")
            for wi in range(40):
                nc.tensor.matmul(
                    warm_ps[:, (wi % 8) * BS : (wi % 8) * BS + BS],
                    lhsT=ones_t[:],
                    rhs=ones_t[:],
                    start=True,
                    stop=True,
                )

            state = {}

            def emit_mm1(c):
                ch = chunks[c]
                s_ps = ps_s.tile([BS, CAP * BS], f32, tag="s", name="s_ps")
                for off, qoffs, w, j in ch["mm1s"]:
                    if len(qoffs) == 2:
                        base = state["qt"][:, qoffs[0] : qoffs[0] + BS]
                        rhs = bass.AP(
                            tensor=base.tensor,
                            offset=base.offset,
                            ap=[base.ap[0], [qoffs[1] - qoffs[0], 2], [1, BS]],
                        )
                    else:
                        rhs = state["qt"][:, qoffs[0] : qoffs[0] + w]
                    nc.tensor.matmul(
                        s_ps[:, off : off + w],
                        lhsT=state["kt"][:, j * BS : (j + 1) * BS],
                        rhs=rhs,
                        start=True,
                        stop=True,
                    )
                e_t = epool.tile([BS, CAP * BS], f16, tag="e")
                nc.scalar.activation(
                    e_t[:, : ch["used"]], s_ps[:, : ch["used"]], AF.Exp, scale=SCALE
                )
                state[("e", c)] = e_t
                state[("s", c)] = s_ps

            def emit_dmm(c, on_pool):
                ch = chunks[c]
                used = ch["used"]
                e_t = state[("e", c)]
                s_ps = state.pop(("s", c))
                # denominators via ones-matmul, replicated to all partitions,
                # written into the (dead) score tile's PSUM banks: 512-col
                # pieces (a PSUM accumulation group must stay within one bank)
                d_in = s_ps
                for half in range(0, used, 4 * BS):
                    hi = min(used, half + 4 * BS)
                    nc.tensor.matmul(
                        d_in[:, half:hi],
                        lhsT=ones_t[:],
                        rhs=e_t[:, half:hi],
                        start=True,
                        stop=True,
                    )
                eh_t = ehpool.tile([BS, CAP * BS], f16, tag="eh")
                nc.vector._custom_dve(
                    recip_mul,
                    out=eh_t[:, :used],
                    in0=e_t[:, :used],
                    in1=d_in[:, :used],
                    s0=_RC["s0"],
                    s1=_RC["s1"],
                    imm2=_RC["imm2"],
                )
                state[("eh", c)] = eh_t

            def emit_mm2(c):
                ch = chunks[c]
                eh_t = state.pop(("eh", c))
                state.pop(("e", c))
                if ch["open_group"]:
                    state["o_ps"] = ps_o.tile(
                        [D, GR * BS], f32, tag="o", name="o_ps"
                    )
                    state["o_started"] = False
                    for i in ch["empty_rows"]:
                        nc.vector.memset(
                            state["o_ps"][:, (i % GR) * BS : (i % GR + 1) * BS], 0.0
                        )
                o_ps = state["o_ps"]
                for out_col, w, off, j, _ft in ch["mm2s"]:
                    # One start=True per group tile (marks the whole bank
                    # pending-zero); later matmuls overwrite their first-touch
                    # bytes and accumulate elsewhere. skip_group_check
                    # silences the sim's one-open-group-per-region tracker.
                    nc.tensor.matmul(
                        o_ps[:, out_col : out_col + w],
                        lhsT=state["v"][:, j * BS : (j + 1) * BS],
                        rhs=eh_t[:, off : off + w],
                        start=not state["o_started"],
                        stop=ch["close_group"],
                        skip_group_check=True,
                    )
                    state["o_started"] = True
                if ch["close_group"]:
                    g = ch["g"]
                    o_sb = outpool.tile([D, GR * BS], f16, tag="osb")
                    # alternate drains between ACT and DVE
                    if (state["h"] * 4 + g) % 2 == 0:
                        nc.scalar.copy(o_sb[:], o_ps[:])
                    else:
                        nc.vector.tensor_scalar_mul(o_sb[:], o_ps[:], 1.0)
                    nc.sync.dma_start(
                        out=ot_d[state["h"], :, g * GR * BS : (g + 1) * GR * BS],
                        in_=o_sb[:],
                    )

            # prefetch every head's inputs up front (SP queue runs ahead)
            intiles = []
            for h in range(HPC):
                qt_t = heads.tile([D, S], f16, tag="qt")
                nc.sync.dma_start(out=qt_t[:], in_=qt_d[h])
                kt_t = heads.tile([D, S], f16, tag="kt")
                nc.sync.dma_start(out=kt_t[:], in_=kt_d[h])
                v_t = heads.tile([BS, NB * BS], f16, tag="v")
                nc.sync.dma_start(out=v_t[:], in_=v_d[h])
                intiles.append((qt_t, kt_t, v_t))

            cglob = 0
            for h in range(HPC):
                state["qt"], state["kt"], state["v"] = intiles[h]
                state["h"] = h

                emit_mm1(0)
                if n > 1:
                    emit_mm1(1)
                for c in range(n):
                    emit_dmm(c, on_pool=False)
                    cglob += 1
                    if c + 2 < n:
                        emit_mm1(c + 2)
                    if c >= 3:
                        emit_mm2(c - 3)
                for c in (n - 3, n - 2, n - 1):
                    if c >= 0:
                        emit_mm2(c)

    nc.finalize()
    return nc


_CACHE = {}


def _get_program(mask):
    key = np.asarray(mask).astype(bool).tobytes()
    if key not in _CACHE:
        _CACHE[key] = _build(mask)
    return _CACHE[key]


def _shard_inputs(query, key, value):
    q = np.ascontiguousarray(query, dtype=np.float32).reshape(N_HEADS, S, D)
    k = np.ascontiguousarray(key, dtype=np.float32).reshape(N_HEADS, S, D)
    v = np.ascontiguousarray(value, dtype=np.float32).reshape(N_HEADS, S, D)
    qt = np.ascontiguousarray(q.transpose(0, 2, 1).astype(np.float16))  # (32, D, S)
    kt = np.ascontiguousarray(k.transpose(0, 2, 1).astype(np.float16))
    v16 = np.ascontiguousarray(
        v.reshape(N_HEADS, NB, BS, D).transpose(0, 2, 1, 3).astype(np.float16)
    ).reshape(N_HEADS, BS, NB * BS)
    in_maps = []
    for c in range(N_CORES):
        sl = slice(c * HPC, (c + 1) * HPC)
        in_maps.append(
            {
                "qt": np.ascontiguousarray(qt[sl]),
                "kt": np.ascontiguousarray(kt[sl]),
                "v": np.ascontiguousarray(v16[sl]),
            }
        )
    return in_maps


def _unshard_output(results):
    ot = np.concatenate([r["ot"] for r in results], axis=0)  # (32, D, S) f16
    out = ot.astype(np.float32).transpose(0, 2, 1).reshape(B, H, S, D)
    return np.ascontiguousarray(out)


def kernel(query, key, value, block_mask, block_size, _trace=False):
    from concourse.bass_utils import run_bass_kernel_spmd

    assert int(block_size) == BS
    nc = _get_program(block_mask)
    in_maps = _shard_inputs(query, key, value)
    res = run_bass_kernel_spmd(nc, in_maps, core_ids=list(range(N_CORES)), trace=_trace)
    out = _unshard_output(res.results)
    if _trace:
        return out, res
    return out


# revision 45
# speedup vs baseline: 1.8803x; 1.0072x over previous
"""Block-sparse attention TRN2 kernel (8 NeuronCores, SPMD over batch*heads).

Contract: kernel(**inputs) takes FULL unsharded inputs
  query/key/value: (2, 16, 2048, 128) f32, block_mask: (16, 16) bool,
  block_size: 128
and returns the FULL (2, 16, 2048, 128) f32 output.

Math per (b, h): for each 128x128 block pair (i, j) with block_mask[i, j]:
  A_ij = softmax(Q_i K_j^T / sqrt(128)) (softmax per block over k, no
  cross-block merge), O_i = sum_j A_ij V_j.

Device pipeline ([k, q] orientation; all matmul operands f16):
  Blocks are ordered by (row-group g = i//4, key block j, i) and packed
  8-per-chunk into [128, 1024] PSUM score tiles. The (g, j, i) order
  makes same-j blocks adjacent, so MM1 batches consecutive-i runs into
  single matmuls and pairs leftover singles via 3-level APs (fewer,
  longer PE instructions -> less per-matmul overhead, better p-state).
  Per chunk:
    MM1   S^T = KT_j^T @ QT_i per run (f16, PSUM f32)
    exp   one ACT op per chunk (PSUM f32 -> SBUF f16)
    d     ones[128,128] @ E -> denominators replicated across all 128
          partitions, written into the dead score tile's PSUM banks
    rmul  ONE fused custom-DVE op eh = E * approx_recip(d)
    MM2   O_i^T += V_j^T.T @ eh, batched over uniform-flag row runs,
          accumulated in a per-row-group [128, 512] PSUM tile (4 rows),
          drained to f16 via ACT when the group's last chunk completes.
  Emission is software-pipelined (MM1(c+2) / d(c)+rmul(c) / MM2(c-3))
  so no engine stream ever sits directly behind a cross-engine dep.
  Host does f16 packing and the final O^T -> O transpose.
"""

import math

import numpy as np

_RECIP_MUL = None


def _register_recip_mul():
    """Register a fused out = in0 * (1/in1) custom DVE op (one ~1 elem/cycle
    DVE pass; hardware has no tensor_tensor divide). Seed + one Newton pass,
    ~0.2% max rel err on the reciprocal."""
    global _RECIP_MUL
    if _RECIP_MUL is not None:
        return _RECIP_MUL
    import concourse.dve_ops as dve_ops

    NAME = "RECIP_MUL_ANT"
    for op in dve_ops.OPS:
        if op.name == NAME:
            _RECIP_MUL = op
            return op
    from concourse.dve_spec import AluOp, Bin, C0, C1, Spec, Src0, Src1, _has_src1, lower
    from concourse.dve_uop import DveOpSpec

    _not_x = Bin(AluOp.BITWISE_NOT, Src1, Src1)
    _y0 = _not_x * C0

    def _ref(in0, in1, c0, c1, c2):
        not_x = (~in1.astype(np.float32).view(np.int32)).view(np.float32)
        y0 = not_x * c0
        return (in0 * y0) * (c1 - in1 * y0)

    spec = Spec(body=(Src0 * _y0) * (C1 - Src1 * _y0), reference=_ref)
    row = dve_ops._CUSTOM_DVE_ROW_BASE + len(dve_ops.OPS)
    shas = {}
    for ver in ("v3", "v4"):
        s = DveOpSpec(
            name=NAME, opcode=row, uops=lower(spec, ver=ver), rd1_en=_has_src1(spec)
        )
        shas[ver] = s.sha(ver)
    op = dve_ops.DveOp(NAME, spec, subdim=False, uops_sha=shas)
    dve_ops.OPS.append(op)
    dve_ops.CUSTOM_DVE_SPECS[NAME] = spec
    dve_ops._SUB_OPCODE_FOR_NAME[NAME] = row
    _RECIP_MUL = op
    return op


B, H, S, D = 2, 16, 2048, 128
BS = 128
NB = S // BS
N_CORES = 8
N_HEADS = B * H
HPC = N_HEADS // N_CORES  # heads per core
CAP = 8  # blocks per chunk (8 * 128 = 1024 cols = 2 PSUM banks)
GR = 4  # rows per output group (4 * 128 f32 = one PSUM bank)
SCALE = 1.0 / math.sqrt(float(D))


def _plan(mask):
    """Group-major schedule.

    Returns a list of chunk dicts:
      g      row group (i // GR)
      used   columns used in the [128, CAP*BS] score tile
      mm1s   (off, [qoff] | [qoff1, qoff2], width, j)
      mm2s   (out_col, width, off, j, start, stop)
      open_group / close_group: bool (first / last chunk of the group)
      empty_rows: rows of g with no active blocks (only on open chunks)
    """
    mask = np.asarray(mask).astype(bool)
    assert mask.shape == (NB, NB)
    minj = {i: None for i in range(NB)}
    maxj = {i: None for i in range(NB)}
    for i in range(NB):
        js = np.flatnonzero(mask[i])
        if len(js):
            minj[i], maxj[i] = int(js[0]), int(js[-1])
    chunks = []
    for g in range(NB // GR):
        rows = range(g * GR, (g + 1) * GR)
        entries = []  # (j, i)
        for j in range(NB):
            for i in rows:
                if mask[i, j]:
                    entries.append((j, i))
        if not entries:
            continue
        # items per j: maximal consecutive-i runs
        items = []  # (j, i0, ln), chunk-orderable
        for j in range(NB):
            ii = [i for i in rows if mask[i, j]]
            k = 0
            while k < len(ii):
                ln = 1
                while k + ln < len(ii) and ii[k + ln] == ii[k] + ln:
                    ln += 1
                items.append((j, ii[k], ln))
                k += ln
        # chunk the blocks, keeping each j's items in one chunk so same-j
        # singles can pair into one 256-col matmul via a 3-level AP
        jgroups = {}
        for j, i0, ln in items:
            jgroups.setdefault(j, []).append((j, i0, ln))
        per_chunk = [[]]
        room = CAP
        for j in sorted(jgroups):
            jsz = sum(it[2] for it in jgroups[j])
            if jsz > room:
                per_chunk.append([])
                room = CAP
            per_chunk[-1].extend(jgroups[j])
            room -= jsz
        nch = len(per_chunk)
        for ci in range(nch):
            citems = per_chunk[ci]
            # order: per j, singles first (paired), then longer runs
            ordered = []
            for j in sorted({it[0] for it in citems}):
                sing = [it for it in citems if it[0] == j and it[2] == 1]
                long = [it for it in citems if it[0] == j and it[2] > 1]
                ordered.extend(sing)
                ordered.extend(long)
            mm1s = []
            mm2s = []
            off = 0
            a = 0
            while a < len(ordered):
                j, i0, ln = ordered[a]
                if (
                    ln == 1
                    and a + 1 < len(ordered)
                    and ordered[a + 1][0] == j
                    and ordered[a + 1][2] == 1
                    and off % (4 * BS) != 3 * BS
                ):
                    j2, i2, _ = ordered[a + 1]
                    mm1s.append((off, [i0 * BS, i2 * BS], 2 * BS, j))
                    for bi, ii_ in enumerate((i0, i2)):
                        mm2s.append(
                            ((ii_ - g * GR) * BS, BS, off + bi * BS, j, minj[ii_] == j)
                        )
                    off += 2 * BS
                    a += 2
                    continue
                # run (or lone single): split MM1 at 512-col bank boundaries
                ro, rq, rw = off, i0 * BS, ln * BS
                while rw > 0:
                    w = min(rw, 4 * BS - ro % (4 * BS))
                    mm1s.append((ro, [rq], w, j))
                    ro += w
                    rq += w
                    rw -= w
                # MM2: split run where first-touch status flips (a matmul's
                # bytes must be uniformly overwrite-pending or accumulate)
                s0 = 0
                while s0 < ln:
                    ft = minj[i0 + s0] == j
                    s1 = s0 + 1
                    while s1 < ln and (minj[i0 + s1] == j) == ft:
                        s1 += 1
                    mm2s.append(
                        ((i0 + s0 - g * GR) * BS, (s1 - s0) * BS, off + s0 * BS, j, ft)
                    )
                    s0 = s1
                off += ln * BS
                a += 1
            chunks.append(
                dict(
                    g=g,
                    used=off,
                    mm1s=mm1s,
                    mm2s=mm2s,
                    open_group=(ci == 0),
                    close_group=(ci == nch - 1),
                    empty_rows=[
                        i for i in rows if minj[i] is None
                    ]
                    if ci == 0
                    else [],
                )
            )
    return chunks


def _build(mask):
    import concourse.bacc as bacc
    import concourse.bass as bass
    import concourse.tile as tile
    from concourse import mybir
    from concourse.dve_ops import RECIP_APPROX_FAST_CONSTS as _RC

    f32 = mybir.dt.float32
    f16 = mybir.dt.float16
    AF = mybir.ActivationFunctionType

    recip_mul = _register_recip_mul()
    chunks = _plan(mask)
    n = len(chunks)

    nc = bacc.Bacc(
        "TRN2",
        target_bir_lowering=False,
        debug=False,
        enable_asserts=False,
        num_devices=N_CORES,
    )
    qt_d = nc.dram_tensor("qt", (HPC, D, S), f16, kind="ExternalInput").ap()
    kt_d = nc.dram_tensor("kt", (HPC, D, S), f16, kind="ExternalInput").ap()
    v_d = nc.dram_tensor("v", (HPC, BS, NB * BS), f16, kind="ExternalInput").ap()
    ot_d = nc.dram_tensor("ot", (HPC, D, S), f16, kind="ExternalOutput").ap()

    with tile.TileContext(nc) as tc:
        with (
            tc.tile_pool(name="heads", bufs=HPC) as heads,
            tc.tile_pool(name="const", bufs=1) as const,
            tc.tile_pool(name="e", bufs=6) as epool,
            tc.tile_pool(name="eh", bufs=6) as ehpool,
            tc.tile_pool(name="outp", bufs=4) as outpool,
            tc.tile_pool(name="ps_s", bufs=3, space="PSUM") as ps_s,
            tc.tile_pool(name="ps_o", bufs=2, space="PSUM") as ps_o,
        ):
            ones_t = const.tile([BS, BS], f16)
            nc.vector.memset(ones_t[:], 1.0)

            # PE warmup while the first input DMAs land: keeps the tensor
            # engine busy from t=0 so it ramps to full p-state
            warm_ps = ps_s.tile([BS, CAP * BS], f32, tag="s", name="warm_ps")
            for wi in range(40):
                nc.tensor.matmul(
                    warm_ps[:, (wi % 8) * BS : (wi % 8) * BS + BS],
                    lhsT=ones_t[:],
                    rhs=ones_t[:],
                    start=True,
                    stop=True,
                )

            state = {}

            def emit_mm1(c):
                ch = chunks[c]
                s_ps = ps_s.tile([BS, CAP * BS], f32, tag="s", name="s_ps")
                for off, qoffs, w, j in ch["mm1s"]:
                    if len(qoffs) == 2:
                        base = state["qt"][:, qoffs[0] : qoffs[0] + BS]
                        rhs = bass.AP(
                            tensor=base.tensor,
                            offset=base.offset,
                            ap=[base.ap[0], [qoffs[1] - qoffs[0], 2], [1, BS]],
                        )
                    else:
                        rhs = state["qt"][:, qoffs[0] : qoffs[0] + w]
                    nc.tensor.matmul(
                        s_ps[:, off : off + w],
                        lhsT=state["kt"][:, j * BS : (j + 1) * BS],
                        rhs=rhs,
                        start=True,
                        stop=True,
                    )
                e_t = epool.tile([BS, CAP * BS], f16, tag="e")
                nc.scalar.activation(
                    e_t[:, : ch["used"]], s_ps[:, : ch["used"]], AF.Exp, scale=SCALE
                )
                state[("e", c)] = e_t
                state[("s", c)] = s_ps

            def emit_dmm(c):
                ch = chunks[c]
                used = ch["used"]
                e_t = state[("e", c)]
                s_ps = state.pop(("s", c))
                # denominators via ones-matmul, replicated to all partitions,
                # written into the (dead) score tile's PSUM banks: 512-col
                # pieces (a PSUM accumulation group must stay within one bank)
                d_in = s_ps
                for half in range(0, used, 4 * BS):
                    hi = min(used, half + 4 * BS)
                    nc.tensor.matmul(
                        d_in[:, half:hi],
                        lhsT=ones_t[:],
                        rhs=e_t[:, half:hi],
                        start=True,
                        stop=True,
                    )
                eh_t = ehpool.tile([BS, CAP * BS], f16, tag="eh")
                nc.vector._custom_dve(
                    recip_mul,
                    out=eh_t[:, :used],
                    in0=e_t[:, :used],
                    in1=d_in[:, :used],
                    s0=_RC["s0"],
                    s1=_RC["s1"],
                    imm2=_RC["imm2"],
                )
                state[("eh", c)] = eh_t

            def emit_mm2(c):
                ch = chunks[c]
                eh_t = state.pop(("eh", c))
                state.pop(("e", c))
                if ch["open_group"]:
                    state["o_ps"] = ps_o.tile(
                        [D, GR * BS], f32, tag="o", name="o_ps"
                    )
                    state["o_started"] = False
                    for i in ch["empty_rows"]:
                        nc.vector.memset(
                            state["o_ps"][:, (i % GR) * BS : (i % GR + 1) * BS], 0.0
                        )
                o_ps = state["o_ps"]
                for out_col, w, off, j, _ft in ch["mm2s"]:
                    # One start=True per group tile (marks the whole bank
                    # pending-zero); later matmuls overwrite their first-touch
                    # bytes and accumulate elsewhere. skip_group_check
                    # silences the sim's one-open-group-per-region tracker.
                    nc.tensor.matmul(
                        o_ps[:, out_col : out_col + w],
                        lhsT=state["v"][:, j * BS : (j + 1) * BS],
                        rhs=eh_t[:, off : off + w],
                        start=not state["o_started"],
                        stop=ch["close_group"],
                        skip_group_check=True,
                    )
                    state["o_started"] = True
                if ch["close_group"]:
                    g = ch["g"]
                    o_sb = outpool.tile([D, GR * BS], f16, tag="osb")
                    # alternate drains between ACT and DVE
                    if (state["h"] * 4 + g) % 2 == 0:
                        nc.scalar.copy(o_sb[:], o_ps[:])
                    else:
                        nc.vector.tensor_scalar_mul(o_sb[:], o_ps[:], 1.0)
                    nc.sync.dma_start(
                        out=ot_d[state["h"], :, g * GR * BS : (g + 1) * GR * BS],
                        in_=o_sb[:],
                    )

            # prefetch every head's inputs up front (SP queue runs ahead)
            intiles = []
            for h in range(HPC):
                qt_t = heads.tile([D, S], f16, tag="qt")
                nc.sync.dma_start(out=qt_t[:], in_=qt_d[h])
                kt_t = heads.tile([D, S], f16, tag="kt")
                nc.sync.dma_start(out=kt_t[:], in_=kt_d[h])
                v_t = heads.tile([BS, NB * BS], f16, tag="v")
                nc.sync.dma_start(out=v_t[:], in_=v_d[h])
                intiles.append((qt_t, kt_t, v_t))

            for h in range(HPC):
                state["qt"], state["kt"], state["v"] = intiles[h]
                state["h"] = h

                emit_mm1(0)
                if n > 1:
                    emit_mm1(1)
                for c in range(n):
                    emit_dmm(c)
                    if c + 2 < n:
                        emit_mm1(c + 2)
                    if c >= 3:
                        emit_mm2(c - 3)
                for c in (n - 3, n - 2, n - 1):
                    if c >= 0:
                        emit_mm2(c)

    nc.finalize()
    return nc


_CACHE = {}


def _get_program(mask):
    key = np.asarray(mask).astype(bool).tobytes()
    if key not in _CACHE:
        _CACHE[key] = _build(mask)
    return _CACHE[key]


def _shard_inputs(query, key, value):
    q = np.ascontiguousarray(query, dtype=np.float32).reshape(N_HEADS, S, D)
    k = np.ascontiguousarray(key, dtype=np.float32).reshape(N_HEADS, S, D)
    v = np.ascontiguousarray(value, dtype=np.float32).reshape(N_HEADS, S, D)
    qt = np.ascontiguousarray(q.transpose(0, 2, 1).astype(np.float16))  # (32, D, S)
    kt = np.ascontiguousarray(k.transpose(0, 2, 1).astype(np.float16))
    v16 = np.ascontiguousarray(
        v.reshape(N_HEADS, NB, BS, D).transpose(0, 2, 1, 3).astype(np.float16)
    ).reshape(N_HEADS, BS, NB * BS)
    in_maps = []
    for c in range(N_CORES):
        sl = slice(c * HPC, (c + 1) * HPC)
        in_maps.append(
            {
                "qt": np.ascontiguousarray(qt[sl]),
                "kt": np.ascontiguousarray(kt[sl]),
                "v": np.ascontiguousarray(v16[sl]),
            }
        )
    return in_maps


def _unshard_output(results):
    ot = np.concatenate([r["ot"] for r in results], axis=0)  # (32, D, S) f16
    out = ot.astype(np.float32).transpose(0, 2, 1).reshape(B, H, S, D)
    return np.ascontiguousarray(out)


def kernel(query, key, value, block_mask, block_size, _trace=False):
    from concourse.bass_utils import run_bass_kernel_spmd

    assert int(block_size) == BS
    nc = _get_program(block_mask)
    in_maps = _shard_inputs(query, key, value)
    res = run_bass_kernel_spmd(nc, in_maps, core_ids=list(range(N_CORES)), trace=_trace)
    out = _unshard_output(res.results)
    if _trace:
        return out, res
    return out


# revision 51
# speedup vs baseline: 1.8860x; 1.0030x over previous
"""Block-sparse attention TRN2 kernel (8 NeuronCores, SPMD over batch*heads).

Contract: kernel(**inputs) takes FULL unsharded inputs
  query/key/value: (2, 16, 2048, 128) f32, block_mask: (16, 16) bool,
  block_size: 128
and returns the FULL (2, 16, 2048, 128) f32 output.

Math per (b, h): for each 128x128 block pair (i, j) with block_mask[i, j]:
  A_ij = softmax(Q_i K_j^T / sqrt(128)) (softmax per block over k, no
  cross-block merge), O_i = sum_j A_ij V_j.

Device pipeline ([k, q] orientation; all matmul operands f16):
  Blocks are ordered by (row-group g = i//4, key block j, i) and packed
  8-per-chunk into [128, 1024] PSUM score tiles. The (g, j, i) order
  makes same-j blocks adjacent, so MM1 batches consecutive-i runs into
  single matmuls and pairs leftover singles via 3-level APs (fewer,
  longer PE instructions -> less per-matmul overhead, better p-state).
  Per chunk:
    MM1   S^T = KT_j^T @ QT_i per run (f16, PSUM f32)
    exp   one ACT op per chunk (PSUM f32 -> SBUF f16)
    d     ones[128,128] @ E -> denominators replicated across all 128
          partitions, written into the dead score tile's PSUM banks
    rmul  ONE fused custom-DVE op eh = E * approx_recip(d)
    MM2   O_i^T += V_j^T.T @ eh, batched over uniform-flag row runs,
          accumulated in a per-row-group [128, 512] PSUM tile (4 rows),
          drained to f16 via ACT when the group's last chunk completes.
  Emission is software-pipelined (MM1(c+2) / d(c)+rmul(c) / MM2(c-3))
  so no engine stream ever sits directly behind a cross-engine dep.
  Host does f16 packing and the final O^T -> O transpose.
"""

import math

import numpy as np

_RECIP_MUL = None


def _register_recip_mul():
    """Register a fused out = in0 * (1/in1) custom DVE op (one ~1 elem/cycle
    DVE pass; hardware has no tensor_tensor divide). Seed + one Newton pass,
    ~0.2% max rel err on the reciprocal."""
    global _RECIP_MUL
    if _RECIP_MUL is not None:
        return _RECIP_MUL
    import concourse.dve_ops as dve_ops

    NAME = "RECIP_MUL_ANT"
    for op in dve_ops.OPS:
        if op.name == NAME:
            _RECIP_MUL = op
            return op
    from concourse.dve_spec import AluOp, Bin, C0, C1, Spec, Src0, Src1, _has_src1, lower
    from concourse.dve_uop import DveOpSpec

    _not_x = Bin(AluOp.BITWISE_NOT, Src1, Src1)
    _y0 = _not_x * C0

    def _ref(in0, in1, c0, c1, c2):
        not_x = (~in1.astype(np.float32).view(np.int32)).view(np.float32)
        y0 = not_x * c0
        return (in0 * y0) * (c1 - in1 * y0)

    spec = Spec(body=(Src0 * _y0) * (C1 - Src1 * _y0), reference=_ref)
    row = dve_ops._CUSTOM_DVE_ROW_BASE + len(dve_ops.OPS)
    shas = {}
    for ver in ("v3", "v4"):
        s = DveOpSpec(
            name=NAME, opcode=row, uops=lower(spec, ver=ver), rd1_en=_has_src1(spec)
        )
        shas[ver] = s.sha(ver)
    op = dve_ops.DveOp(NAME, spec, subdim=False, uops_sha=shas)
    dve_ops.OPS.append(op)
    dve_ops.CUSTOM_DVE_SPECS[NAME] = spec
    dve_ops._SUB_OPCODE_FOR_NAME[NAME] = row
    _RECIP_MUL = op
    return op


B, H, S, D = 2, 16, 2048, 128
BS = 128
NB = S // BS
N_CORES = 8
N_HEADS = B * H
HPC = N_HEADS // N_CORES  # heads per core
CAP = 8  # blocks per chunk (8 * 128 = 1024 cols = 2 PSUM banks)
GR = 4  # rows per output group (4 * 128 f32 = one PSUM bank)
SCALE = 1.0 / math.sqrt(float(D))


def _plan(mask):
    """Group-major schedule.

    Returns a list of chunk dicts:
      g      row group (i // GR)
      used   columns used in the [128, CAP*BS] score tile
      mm1s   (off, [qoff] | [qoff1, qoff2], width, j)
      mm2s   (out_col, width, off, j, start, stop)
      open_group / close_group: bool (first / last chunk of the group)
      empty_rows: rows of g with no active blocks (only on open chunks)
    """
    mask = np.asarray(mask).astype(bool)
    assert mask.shape == (NB, NB)
    minj = {i: None for i in range(NB)}
    maxj = {i: None for i in range(NB)}
    for i in range(NB):
        js = np.flatnonzero(mask[i])
        if len(js):
            minj[i], maxj[i] = int(js[0]), int(js[-1])
    chunks = []
    for g in range(NB // GR):
        rows = range(g * GR, (g + 1) * GR)
        entries = []  # (j, i)
        for j in range(NB):
            for i in rows:
                if mask[i, j]:
                    entries.append((j, i))
        if not entries:
            continue
        # items per j: maximal consecutive-i runs
        items = []  # (j, i0, ln), chunk-orderable
        for j in range(NB):
            ii = [i for i in rows if mask[i, j]]
            k = 0
            while k < len(ii):
                ln = 1
                while k + ln < len(ii) and ii[k + ln] == ii[k] + ln:
                    ln += 1
                items.append((j, ii[k], ln))
                k += ln
        # chunk the blocks, keeping each j's items in one chunk so same-j
        # singles can pair into one 256-col matmul via a 3-level AP
        jgroups = {}
        for j, i0, ln in items:
            jgroups.setdefault(j, []).append((j, i0, ln))
        per_chunk = [[]]
        room = CAP
        for j in sorted(jgroups):
            jsz = sum(it[2] for it in jgroups[j])
            if jsz > room:
                per_chunk.append([])
                room = CAP
            per_chunk[-1].extend(jgroups[j])
            room -= jsz
        nch = len(per_chunk)
        for ci in range(nch):
            citems = per_chunk[ci]
            # order: per j, singles first (paired), then longer runs
            ordered = []
            for j in sorted({it[0] for it in citems}):
                sing = [it for it in citems if it[0] == j and it[2] == 1]
                long = [it for it in citems if it[0] == j and it[2] > 1]
                ordered.extend(sing)
                ordered.extend(long)
            mm1s = []
            mm2s = []
            off = 0
            a = 0
            while a < len(ordered):
                j, i0, ln = ordered[a]
                if (
                    ln == 1
                    and a + 1 < len(ordered)
                    and ordered[a + 1][0] == j
                    and ordered[a + 1][2] == 1
                    and off % (4 * BS) != 3 * BS
                ):
                    j2, i2, _ = ordered[a + 1]
                    mm1s.append((off, [i0 * BS, i2 * BS], 2 * BS, j))
                    for bi, ii_ in enumerate((i0, i2)):
                        mm2s.append(
                            ((ii_ - g * GR) * BS, BS, off + bi * BS, j, minj[ii_] == j)
                        )
                    off += 2 * BS
                    a += 2
                    continue
                # run (or lone single): split MM1 at 512-col bank boundaries
                ro, rq, rw = off, i0 * BS, ln * BS
                while rw > 0:
                    w = min(rw, 4 * BS - ro % (4 * BS))
                    mm1s.append((ro, [rq], w, j))
                    ro += w
                    rq += w
                    rw -= w
                # MM2: split run where first-touch status flips (a matmul's
                # bytes must be uniformly overwrite-pending or accumulate)
                s0 = 0
                while s0 < ln:
                    ft = minj[i0 + s0] == j
                    s1 = s0 + 1
                    while s1 < ln and (minj[i0 + s1] == j) == ft:
                        s1 += 1
                    mm2s.append(
                        ((i0 + s0 - g * GR) * BS, (s1 - s0) * BS, off + s0 * BS, j, ft)
                    )
                    s0 = s1
                off += ln * BS
                a += 1
            chunks.append(
                dict(
                    g=g,
                    used=off,
                    mm1s=mm1s,
                    mm2s=mm2s,
                    open_group=(ci == 0),
                    close_group=(ci == nch - 1),
                    empty_rows=[
                        i for i in rows if minj[i] is None
                    ]
                    if ci == 0
                    else [],
                )
            )
    return chunks


def _build(mask):
    import concourse.bacc as bacc
    import concourse.bass as bass
    import concourse.tile as tile
    from concourse import mybir
    from concourse.dve_ops import RECIP_APPROX_FAST_CONSTS as _RC

    f32 = mybir.dt.float32
    f16 = mybir.dt.float16
    AF = mybir.ActivationFunctionType

    recip_mul = _register_recip_mul()
    chunks = _plan(mask)
    n = len(chunks)

    nc = bacc.Bacc(
        "TRN2",
        target_bir_lowering=False,
        debug=False,
        enable_asserts=False,
        num_devices=N_CORES,
    )
    qt_d = nc.dram_tensor("qt", (HPC, D, S), f16, kind="ExternalInput").ap()
    kt_d = nc.dram_tensor("kt", (HPC, D, S), f16, kind="ExternalInput").ap()
    v_d = nc.dram_tensor("v", (HPC, BS, NB * BS), f16, kind="ExternalInput").ap()
    ones_d = nc.dram_tensor("ones", (BS, BS), f16, kind="ExternalInput").ap()
    ot_d = nc.dram_tensor("ot", (HPC, D, S), f16, kind="ExternalOutput").ap()

    with tile.TileContext(nc) as tc:
        with (
            tc.tile_pool(name="heads", bufs=HPC) as heads,
            tc.tile_pool(name="const", bufs=1) as const,
            tc.tile_pool(name="e", bufs=6) as epool,
            tc.tile_pool(name="eh", bufs=6) as ehpool,
            tc.tile_pool(name="outp", bufs=4) as outpool,
            tc.tile_pool(name="ps_s", bufs=3, space="PSUM") as ps_s,
            tc.tile_pool(name="ps_o", bufs=2, space="PSUM") as ps_o,
        ):
            ones_t = const.tile([BS, BS], f16)
            nc.sync.dma_start(out=ones_t[:], in_=ones_d)

            # PE warmup while the first input DMAs land: keeps the tensor
            # engine busy from t=0 so it ramps to full p-state
            warm_ps = ps_s.tile([BS, CAP * BS], f32, tag="s", name="warm_ps")
            for wi in range(44):
                nc.tensor.matmul(
                    warm_ps[:, (wi % 8) * BS : (wi % 8) * BS + BS],
                    lhsT=ones_t[:],
                    rhs=ones_t[:],
                    start=True,
                    stop=True,
                )

            state = {}
            # flat cross-head chunk stream: the software pipeline never
            # refills at head boundaries
            gchunks = [(h, c) for h in range(HPC) for c in range(n)]
            N = len(gchunks)

            def emit_mm1(gi):
                h, c = gchunks[gi]
                ch = chunks[c]
                qt_t = state["in"][h][0]
                s_ps = ps_s.tile([BS, CAP * BS], f32, tag="s", name="s_ps")
                for off, qoffs, w, j in ch["mm1s"]:
                    if len(qoffs) == 2:
                        base = qt_t[:, qoffs[0] : qoffs[0] + BS]
                        rhs = bass.AP(
                            tensor=base.tensor,
                            offset=base.offset,
                            ap=[base.ap[0], [qoffs[1] - qoffs[0], 2], [1, BS]],
                        )
                    else:
                        rhs = qt_t[:, qoffs[0] : qoffs[0] + w]
                    nc.tensor.matmul(
                        s_ps[:, off : off + w],
                        lhsT=state["in"][h][1][:, j * BS : (j + 1) * BS],
                        rhs=rhs,
                        start=True,
                        stop=True,
                    )
                e_t = epool.tile([BS, CAP * BS], f16, tag="e")
                nc.scalar.activation(
                    e_t[:, : ch["used"]], s_ps[:, : ch["used"]], AF.Exp, scale=SCALE
                )
                state[("e", gi)] = e_t
                state[("s", gi)] = s_ps

            def emit_dmm(gi):
                _h, c = gchunks[gi]
                ch = chunks[c]
                used = ch["used"]
                e_t = state[("e", gi)]
                s_ps = state.pop(("s", gi))
                # denominators via ones-matmul, replicated to all partitions,
                # written into the (dead) score tile's PSUM banks: 512-col
                # pieces (a PSUM accumulation group must stay within one bank)
                d_in = s_ps
                for half in range(0, used, 4 * BS):
                    hi = min(used, half + 4 * BS)
                    nc.tensor.matmul(
                        d_in[:, half:hi],
                        lhsT=ones_t[:],
                        rhs=e_t[:, half:hi],
                        start=True,
                        stop=True,
                    )
                eh_t = ehpool.tile([BS, CAP * BS], f16, tag="eh")
                nc.vector._custom_dve(
                    recip_mul,
                    out=eh_t[:, :used],
                    in0=e_t[:, :used],
                    in1=d_in[:, :used],
                    s0=_RC["s0"],
                    s1=_RC["s1"],
                    imm2=_RC["imm2"],
                )
                state[("eh", gi)] = eh_t

            def emit_mm2(gi):
                h, c = gchunks[gi]
                ch = chunks[c]
                eh_t = state.pop(("eh", gi))
                state.pop(("e", gi))
                if ch["open_group"]:
                    state["o_ps"] = ps_o.tile(
                        [D, GR * BS], f32, tag="o", name="o_ps"
                    )
                    state["o_started"] = False
                    for i in ch["empty_rows"]:
                        nc.vector.memset(
                            state["o_ps"][:, (i % GR) * BS : (i % GR + 1) * BS], 0.0
                        )
                o_ps = state["o_ps"]
                for out_col, w, off, j, _ft in ch["mm2s"]:
                    # One start=True per group tile (marks the whole bank
                    # pending-zero); later matmuls overwrite their first-touch
                    # bytes and accumulate elsewhere. skip_group_check
                    # silences the sim's one-open-group-per-region tracker.
                    nc.tensor.matmul(
                        o_ps[:, out_col : out_col + w],
                        lhsT=state["in"][h][2][:, j * BS : (j + 1) * BS],
                        rhs=eh_t[:, off : off + w],
                        start=not state["o_started"],
                        stop=ch["close_group"],
                        skip_group_check=True,
                    )
                    state["o_started"] = True
                if ch["close_group"]:
                    g = ch["g"]
                    o_sb = outpool.tile([D, GR * BS], f16, tag="osb")
                    # alternate drains between ACT and DVE
                    if (h * 4 + g) % 2 == 0:
                        nc.scalar.copy(o_sb[:], o_ps[:])
                    else:
                        nc.vector.tensor_scalar_mul(o_sb[:], o_ps[:], 1.0)
                    nc.sync.dma_start(
                        out=ot_d[h, :, g * GR * BS : (g + 1) * GR * BS],
                        in_=o_sb[:],
                    )

            # prefetch every head's inputs up front (SP queue runs ahead)
            intiles = []
            for h in range(HPC):
                qt_t = heads.tile([D, S], f16, tag="qt")
                nc.sync.dma_start(out=qt_t[:], in_=qt_d[h])
                kt_t = heads.tile([D, S], f16, tag="kt")
                nc.sync.dma_start(out=kt_t[:], in_=kt_d[h])
                v_t = heads.tile([BS, NB * BS], f16, tag="v")
                nc.sync.dma_start(out=v_t[:], in_=v_d[h])
                intiles.append((qt_t, kt_t, v_t))
            state["in"] = intiles

            emit_mm1(0)
            if N > 1:
                emit_mm1(1)
            for gi in range(N):
                emit_dmm(gi)
                if gi + 2 < N:
                    emit_mm1(gi + 2)
                if gi >= 3:
                    emit_mm2(gi - 3)
            for gi in (N - 3, N - 2, N - 1):
                if gi >= 0:
                    emit_mm2(gi)

    nc.finalize()
    return nc


_CACHE = {}


def _get_program(mask):
    key = np.asarray(mask).astype(bool).tobytes()
    if key not in _CACHE:
        _CACHE[key] = _build(mask)
    return _CACHE[key]


def _shard_inputs(query, key, value):
    q = np.ascontiguousarray(query, dtype=np.float32).reshape(N_HEADS, S, D)
    k = np.ascontiguousarray(key, dtype=np.float32).reshape(N_HEADS, S, D)
    v = np.ascontiguousarray(value, dtype=np.float32).reshape(N_HEADS, S, D)
    qt = np.ascontiguousarray(q.transpose(0, 2, 1).astype(np.float16))  # (32, D, S)
    kt = np.ascontiguousarray(k.transpose(0, 2, 1).astype(np.float16))
    v16 = np.ascontiguousarray(
        v.reshape(N_HEADS, NB, BS, D).transpose(0, 2, 1, 3).astype(np.float16)
    ).reshape(N_HEADS, BS, NB * BS)
    ones = np.ones((BS, BS), dtype=np.float16)
    in_maps = []
    for c in range(N_CORES):
        sl = slice(c * HPC, (c + 1) * HPC)
        in_maps.append(
            {
                "qt": np.ascontiguousarray(qt[sl]),
                "kt": np.ascontiguousarray(kt[sl]),
                "v": np.ascontiguousarray(v16[sl]),
                "ones": ones,
            }
        )
    return in_maps


def _unshard_output(results):
    ot = np.concatenate([r["ot"] for r in results], axis=0)  # (32, D, S) f16
    out = ot.astype(np.float32).transpose(0, 2, 1).reshape(B, H, S, D)
    return np.ascontiguousarray(out)


def kernel(query, key, value, block_mask, block_size, _trace=False):
    from concourse.bass_utils import run_bass_kernel_spmd

    assert int(block_size) == BS
    nc = _get_program(block_mask)
    in_maps = _shard_inputs(query, key, value)
    res = run_bass_kernel_spmd(nc, in_maps, core_ids=list(range(N_CORES)), trace=_trace)
    out = _unshard_output(res.results)
    if _trace:
        return out, res
    return out


# revision 54
# speedup vs baseline: 1.9587x; 1.0385x over previous
"""Block-sparse attention TRN2 kernel (8 NeuronCores, SPMD over batch*heads).

Contract: kernel(**inputs) takes FULL unsharded inputs
  query/key/value: (2, 16, 2048, 128) f32, block_mask: (16, 16) bool,
  block_size: 128
and returns the FULL (2, 16, 2048, 128) f32 output.

Math per (b, h): for each 128x128 block pair (i, j) with block_mask[i, j]:
  A_ij = softmax(Q_i K_j^T / sqrt(128)) (softmax per block over k, no
  cross-block merge), O_i = sum_j A_ij V_j.

Device pipeline ([k, q] orientation; all matmul operands f16):
  Blocks are ordered by (row-group g = i//4, key block j, i) and packed
  8-per-chunk into [128, 1024] PSUM score tiles. The (g, j, i) order
  makes same-j blocks adjacent, so MM1 batches consecutive-i runs into
  single matmuls and pairs leftover singles via 3-level APs (fewer,
  longer PE instructions -> less per-matmul overhead, better p-state).
  Per chunk:
    MM1   S^T = KT_j^T @ QT_i per run (f16, PSUM f32)
    exp   one ACT op per chunk (PSUM f32 -> SBUF f16)
    d     ones[128,128] @ E -> denominators replicated across all 128
          partitions, written into the dead score tile's PSUM banks
    rmul  ONE fused custom-DVE op eh = E * approx_recip(d)
    MM2   O_i^T += V_j^T.T @ eh, batched over uniform-flag row runs,
          accumulated in a per-row-group [128, 512] PSUM tile (4 rows),
          drained to f16 via ACT when the group's last chunk completes.
  Emission is software-pipelined (MM1(c+2) / d(c)+rmul(c) / MM2(c-3))
  so no engine stream ever sits directly behind a cross-engine dep.
  Host does f16 packing and the final O^T -> O transpose.
"""

import math

import numpy as np

_RECIP_MUL = None


def _register_recip_mul():
    """Register a fused out = in0 * (1/in1) custom DVE op (one ~1 elem/cycle
    DVE pass; hardware has no tensor_tensor divide). Seed + one Newton pass,
    ~0.2% max rel err on the reciprocal."""
    global _RECIP_MUL
    if _RECIP_MUL is not None:
        return _RECIP_MUL
    import concourse.dve_ops as dve_ops

    NAME = "RECIP_MUL_ANT"
    for op in dve_ops.OPS:
        if op.name == NAME:
            _RECIP_MUL = op
            return op
    from concourse.dve_spec import AluOp, Bin, C0, C1, Spec, Src0, Src1, _has_src1, lower
    from concourse.dve_uop import DveOpSpec

    _not_x = Bin(AluOp.BITWISE_NOT, Src1, Src1)
    _y0 = _not_x * C0

    def _ref(in0, in1, c0, c1, c2):
        not_x = (~in1.astype(np.float32).view(np.int32)).view(np.float32)
        y0 = not_x * c0
        return (in0 * y0) * (c1 - in1 * y0)

    spec = Spec(body=(Src0 * _y0) * (C1 - Src1 * _y0), reference=_ref)
    row = dve_ops._CUSTOM_DVE_ROW_BASE + len(dve_ops.OPS)
    shas = {}
    for ver in ("v3", "v4"):
        s = DveOpSpec(
            name=NAME, opcode=row, uops=lower(spec, ver=ver), rd1_en=_has_src1(spec)
        )
        shas[ver] = s.sha(ver)
    op = dve_ops.DveOp(NAME, spec, subdim=False, uops_sha=shas)
    dve_ops.OPS.append(op)
    dve_ops.CUSTOM_DVE_SPECS[NAME] = spec
    dve_ops._SUB_OPCODE_FOR_NAME[NAME] = row
    _RECIP_MUL = op
    return op


B, H, S, D = 2, 16, 2048, 128
BS = 128
NB = S // BS
N_CORES = 8
N_HEADS = B * H
HPC = N_HEADS // N_CORES  # heads per core
CAP = 8  # blocks per chunk (8 * 128 = 1024 cols = 2 PSUM banks)
GR = 4  # rows per output group (4 * 128 f32 = one PSUM bank)
SCALE = 1.0 / math.sqrt(float(D))


def _plan(mask):
    """Globally packed group-major schedule.

    Blocks are ordered (row-group g = i//GR, key block j, i) and packed
    j-group-atomically into chunks of <= CAP blocks; chunks may span row
    groups. Returns a list of chunk dicts:
      used     columns used in the [128, CAP*BS] score tile
      mm1s     (off, [qoff] | [qoff1, qoff2], width, j)
      mm2s     (g, out_col, width, off, j)
      closes   row groups whose last block is in this chunk
      empties  {g: [empty rows]} attached to the group's first chunk
    """
    mask = np.asarray(mask).astype(bool)
    assert mask.shape == (NB, NB)
    minj = {i: None for i in range(NB)}
    for i in range(NB):
        js = np.flatnonzero(mask[i])
        if len(js):
            minj[i] = int(js[0])
    # (g, j) item groups in schedule order
    units = []  # (g, j, [(i0, ln), ...])
    for g in range(NB // GR):
        rows = range(g * GR, (g + 1) * GR)
        for j in range(NB):
            ii = [i for i in rows if mask[i, j]]
            runs = []
            k = 0
            while k < len(ii):
                ln = 1
                while k + ln < len(ii) and ii[k + ln] == ii[k] + ln:
                    ln += 1
                runs.append((ii[k], ln))
                k += ln
            if runs:
                units.append((g, j, runs))
    # pack unit-atomically into chunks
    per_chunk = [[]]
    room = CAP
    for u in units:
        usz = sum(ln for _, ln in u[2])
        if usz > room:
            per_chunk.append([])
            room = CAP
        per_chunk[-1].append(u)
        room -= usz
    # last chunk touching each group -> close marker
    last_chunk_of_g = {}
    for ci, cu in enumerate(per_chunk):
        for g, _, _ in cu:
            last_chunk_of_g[g] = ci
    first_chunk_of_g = {}
    for ci, cu in enumerate(per_chunk):
        for g, _, _ in cu:
            first_chunk_of_g.setdefault(g, ci)
    empties = {
        g: [i for i in range(g * GR, (g + 1) * GR) if minj[i] is None]
        for g in range(NB // GR)
    }
    chunks = []
    for ci, cu in enumerate(per_chunk):
        mm1s = []
        mm2s = []
        off = 0
        for g, j, runs in cu:
            # order within the unit: singles first (paired), then long runs
            sing = [r for r in runs if r[1] == 1]
            long = [r for r in runs if r[1] > 1]
            a = 0
            while a < len(sing):
                if a + 1 < len(sing) and off % (4 * BS) != 3 * BS:
                    (i1, _), (i2, _) = sing[a], sing[a + 1]
                    mm1s.append((off, [i1 * BS, i2 * BS], 2 * BS, j))
                    mm2s.append((g, (i1 % GR) * BS, BS, off, j))
                    mm2s.append((g, (i2 % GR) * BS, BS, off + BS, j))
                    off += 2 * BS
                    a += 2
                else:
                    i1 = sing[a][0]
                    mm1s.append((off, [i1 * BS], BS, j))
                    mm2s.append((g, (i1 % GR) * BS, BS, off, j))
                    off += BS
                    a += 1
            for i0, ln in long:
                ro, rq, rw = off, i0 * BS, ln * BS
                while rw > 0:
                    w = min(rw, 4 * BS - ro % (4 * BS))
                    mm1s.append((ro, [rq], w, j))
                    ro += w
                    rq += w
                    rw -= w
                # MM2: split run where first-touch status flips
                s0 = 0
                while s0 < ln:
                    ft = minj[i0 + s0] == j
                    s1 = s0 + 1
                    while s1 < ln and (minj[i0 + s1] == j) == ft:
                        s1 += 1
                    mm2s.append(
                        (g, ((i0 + s0) % GR) * BS, (s1 - s0) * BS, off + s0 * BS, j)
                    )
                    s0 = s1
                off += ln * BS
        chunks.append(
            dict(
                used=off,
                mm1s=mm1s,
                mm2s=mm2s,
                closes=[g for g in last_chunk_of_g if last_chunk_of_g[g] == ci],
                empties={
                    g: empties[g]
                    for g in first_chunk_of_g
                    if first_chunk_of_g[g] == ci and empties[g]
                },
            )
        )
    return chunks


def _build(mask):
    import concourse.bacc as bacc
    import concourse.bass as bass
    import concourse.tile as tile
    from concourse import mybir
    from concourse.dve_ops import RECIP_APPROX_FAST_CONSTS as _RC

    f32 = mybir.dt.float32
    f16 = mybir.dt.float16
    AF = mybir.ActivationFunctionType

    recip_mul = _register_recip_mul()
    chunks = _plan(mask)
    n = len(chunks)

    nc = bacc.Bacc(
        "TRN2",
        target_bir_lowering=False,
        debug=False,
        enable_asserts=False,
        num_devices=N_CORES,
    )
    qt_d = nc.dram_tensor("qt", (HPC, D, S), f16, kind="ExternalInput").ap()
    kt_d = nc.dram_tensor("kt", (HPC, D, S), f16, kind="ExternalInput").ap()
    v_d = nc.dram_tensor("v", (HPC, BS, NB * BS), f16, kind="ExternalInput").ap()
    ones_d = nc.dram_tensor("ones", (BS, BS), f16, kind="ExternalInput").ap()
    ot_d = nc.dram_tensor("ot", (HPC, D, S), f16, kind="ExternalOutput").ap()

    with tile.TileContext(nc) as tc:
        with (
            tc.tile_pool(name="heads", bufs=HPC) as heads,
            tc.tile_pool(name="const", bufs=1) as const,
            tc.tile_pool(name="e", bufs=6) as epool,
            tc.tile_pool(name="eh", bufs=6) as ehpool,
            tc.tile_pool(name="outp", bufs=4) as outpool,
            tc.tile_pool(name="ps_s", bufs=3, space="PSUM") as ps_s,
            tc.tile_pool(name="ps_o", bufs=2, space="PSUM") as ps_o,
        ):
            ones_t = const.tile([BS, BS], f16)
            nc.sync.dma_start(out=ones_t[:], in_=ones_d)

            # PE warmup while the first input DMAs land: keeps the tensor
            # engine busy from t=0 so it ramps to full p-state
            warm_ps = ps_s.tile([BS, CAP * BS], f32, tag="s", name="warm_ps")
            for wi in range(44):
                nc.tensor.matmul(
                    warm_ps[:, (wi % 8) * BS : (wi % 8) * BS + BS],
                    lhsT=ones_t[:],
                    rhs=ones_t[:],
                    start=True,
                    stop=True,
                )

            state = {}
            # flat cross-head chunk stream: the software pipeline never
            # refills at head boundaries
            gchunks = [(h, c) for h in range(HPC) for c in range(n)]
            N = len(gchunks)

            def emit_mm1(gi):
                h, c = gchunks[gi]
                ch = chunks[c]
                qt_t = state["in"][h][0]
                s_ps = ps_s.tile([BS, CAP * BS], f32, tag="s", name="s_ps")
                for off, qoffs, w, j in ch["mm1s"]:
                    if len(qoffs) == 2:
                        base = qt_t[:, qoffs[0] : qoffs[0] + BS]
                        rhs = bass.AP(
                            tensor=base.tensor,
                            offset=base.offset,
                            ap=[base.ap[0], [qoffs[1] - qoffs[0], 2], [1, BS]],
                        )
                    else:
                        rhs = qt_t[:, qoffs[0] : qoffs[0] + w]
                    nc.tensor.matmul(
                        s_ps[:, off : off + w],
                        lhsT=state["in"][h][1][:, j * BS : (j + 1) * BS],
                        rhs=rhs,
                        start=True,
                        stop=True,
                    )
                e_t = epool.tile([BS, CAP * BS], f16, tag="e")
                nc.scalar.activation(
                    e_t[:, : ch["used"]], s_ps[:, : ch["used"]], AF.Exp, scale=SCALE
                )
                state[("e", gi)] = e_t
                state[("s", gi)] = s_ps

            def emit_dmm(gi):
                _h, c = gchunks[gi]
                ch = chunks[c]
                used = ch["used"]
                e_t = state[("e", gi)]
                s_ps = state.pop(("s", gi))
                # denominators via ones-matmul, replicated to all partitions,
                # written into the (dead) score tile's PSUM banks: 512-col
                # pieces (a PSUM accumulation group must stay within one bank)
                d_in = s_ps
                for half in range(0, used, 4 * BS):
                    hi = min(used, half + 4 * BS)
                    nc.tensor.matmul(
                        d_in[:, half:hi],
                        lhsT=ones_t[:],
                        rhs=e_t[:, half:hi],
                        start=True,
                        stop=True,
                    )
                eh_t = ehpool.tile([BS, CAP * BS], f16, tag="eh")
                nc.vector._custom_dve(
                    recip_mul,
                    out=eh_t[:, :used],
                    in0=e_t[:, :used],
                    in1=d_in[:, :used],
                    s0=_RC["s0"],
                    s1=_RC["s1"],
                    imm2=_RC["imm2"],
                )
                state[("eh", gi)] = eh_t

            def emit_mm2(gi):
                h, c = gchunks[gi]
                ch = chunks[c]
                eh_t = state.pop(("eh", gi))
                state.pop(("e", gi))
                o_tiles = state["o_tiles"]
                for g, out_col, w, off, j in ch["mm2s"]:
                    if (h, g) not in o_tiles:
                        t = ps_o.tile([D, GR * BS], f32, tag="o", name="o_ps")
                        o_tiles[(h, g)] = [t, False]
                        for i in ch["empties"].get(g, ()):
                            nc.vector.memset(
                                t[:, (i % GR) * BS : (i % GR + 1) * BS], 0.0
                            )
                    ent = o_tiles[(h, g)]
                    # One start=True per group tile (marks the whole bank
                    # pending-zero); later matmuls overwrite their first-touch
                    # bytes and accumulate elsewhere. skip_group_check
                    # silences the sim's one-open-group-per-region tracker.
                    nc.tensor.matmul(
                        ent[0][:, out_col : out_col + w],
                        lhsT=state["in"][h][2][:, j * BS : (j + 1) * BS],
                        rhs=eh_t[:, off : off + w],
                        start=not ent[1],
                        stop=g in ch["closes"],
                        skip_group_check=True,
                    )
                    ent[1] = True
                for g in ch["closes"]:
                    o_ps = o_tiles.pop((h, g))[0]
                    o_sb = outpool.tile([D, GR * BS], f16, tag="osb")
                    # alternate drains between ACT and DVE
                    if (h * 4 + g) % 2 == 0:
                        nc.scalar.copy(o_sb[:], o_ps[:])
                    else:
                        nc.vector.tensor_scalar_mul(o_sb[:], o_ps[:], 1.0)
                    nc.sync.dma_start(
                        out=ot_d[h, :, g * GR * BS : (g + 1) * GR * BS],
                        in_=o_sb[:],
                    )

            # prefetch every head's inputs up front (SP queue runs ahead)
            intiles = []
            for h in range(HPC):
                qt_t = heads.tile([D, S], f16, tag="qt")
                nc.sync.dma_start(out=qt_t[:], in_=qt_d[h])
                kt_t = heads.tile([D, S], f16, tag="kt")
                nc.sync.dma_start(out=kt_t[:], in_=kt_d[h])
                v_t = heads.tile([BS, NB * BS], f16, tag="v")
                nc.sync.dma_start(out=v_t[:], in_=v_d[h])
                intiles.append((qt_t, kt_t, v_t))
            state["in"] = intiles
            state["o_tiles"] = {}

            emit_mm1(0)
            if N > 1:
                emit_mm1(1)
            for gi in range(N):
                emit_dmm(gi)
                if gi + 2 < N:
                    emit_mm1(gi + 2)
                if gi >= 3:
                    emit_mm2(gi - 3)
            for gi in (N - 3, N - 2, N - 1):
                if gi >= 0:
                    emit_mm2(gi)

    nc.finalize()
    return nc


_CACHE = {}


def _get_program(mask):
    key = np.asarray(mask).astype(bool).tobytes()
    if key not in _CACHE:
        _CACHE[key] = _build(mask)
    return _CACHE[key]


def _shard_inputs(query, key, value):
    q = np.ascontiguousarray(query, dtype=np.float32).reshape(N_HEADS, S, D)
    k = np.ascontiguousarray(key, dtype=np.float32).reshape(N_HEADS, S, D)
    v = np.ascontiguousarray(value, dtype=np.float32).reshape(N_HEADS, S, D)
    qt = np.ascontiguousarray(q.transpose(0, 2, 1).astype(np.float16))  # (32, D, S)
    kt = np.ascontiguousarray(k.transpose(0, 2, 1).astype(np.float16))
    v16 = np.ascontiguousarray(
        v.reshape(N_HEADS, NB, BS, D).transpose(0, 2, 1, 3).astype(np.float16)
    ).reshape(N_HEADS, BS, NB * BS)
    ones = np.ones((BS, BS), dtype=np.float16)
    in_maps = []
    for c in range(N_CORES):
        sl = slice(c * HPC, (c + 1) * HPC)
        in_maps.append(
            {
                "qt": np.ascontiguousarray(qt[sl]),
                "kt": np.ascontiguousarray(kt[sl]),
                "v": np.ascontiguousarray(v16[sl]),
                "ones": ones,
            }
        )
    return in_maps


def _unshard_output(results):
    ot = np.concatenate([r["ot"] for r in results], axis=0)  # (32, D, S) f16
    out = ot.astype(np.float32).transpose(0, 2, 1).reshape(B, H, S, D)
    return np.ascontiguousarray(out)


def kernel(query, key, value, block_mask, block_size, _trace=False):
    from concourse.bass_utils import run_bass_kernel_spmd

    assert int(block_size) == BS
    nc = _get_program(block_mask)
    in_maps = _shard_inputs(query, key, value)
    res = run_bass_kernel_spmd(nc, in_maps, core_ids=list(range(N_CORES)), trace=_trace)
    out = _unshard_output(res.results)
    if _trace:
        return out, res
    return out
